# revision 40
# baseline (speedup 1.0000x reference)
"""GCN (4x GCNConv + eval BN + ReLU, global mean pool, 2-layer MLP head) on 8
Trainium2 NeuronCores via Bass/Tile.

Sharding: data-parallel over graphs. 4096 graphs -> 8 cores x 512 contiguous
graphs (batch is sorted). Within a core the 512 graphs form 4 pool groups of
128 graphs; each group's nodes are padded to a multiple of 128 rows so pooling
blocks align with node blocks. Edges live on the core owning their dst node.

Per layer (all on device):
  tt = dinv * (h_local @ W_l)           per-core shard, f16 table
  AllGather tt across the 8 cores       (the only collective)
  S[v]   = sum_{e: dst=v} tt[src_e] + tt[v]      (one-hot scatter matmuls,
                                                  PURE 0/1 one-hots + a plain
                                                  identity for the self loop)
  agg[v] = dinv[v] * S[v]                        (per-column scale, folded
                                                  into the epilogue)
  h = BN_l(relu(agg + b_l))
Key design points vs the original baseline:

* Edge rows fetched with dma_gather (SWDGE) by int16 index, split by table
  QUARTER. Gather queue assignment is round-robin over BASS_GCN_QUEUES
  (default "0,1"): queue q runs on Q7 core pair (2q, 2q+1), and q0/q1 rings
  are measurably faster than q2/q3.
* num_idxs registers are hoisted (one MOVE per distinct value, not one per
  gather instruction).
* The one-hot scatter matrices are built with a SINGLE is_equal op per
  (group, quarter) — the old per-pair dstw MULTIPLY op is gone. The symmetric
  normalization's dst factor dinv[dst] is applied post-sum as a per-column
  multiply of the PSUM block against a host-precomputed dinvrow constant.
  The self-loop term is a plain constant-identity matmul (table rows already
  carry dinv[src]).
* Padding nodes carry dinv=0 in the t-build scale, so their table rows are
  exactly zero in every layer.
* Layers 0-2 run the scatter matmul "flipped" (aggT [h, node] in PSUM);
  layer 3 runs node-major so pooling sees node-major h.
* T_full (the AllGather output) is allocated in the Shared DRAM space --
  HBM-HBM AllGather is faster with Shared outputs.

All data-dependent structure is precomputed host-side into per-core meta
arrays; the chunk layout is maxed over cores so the device program is
identical across cores (SPMD).
"""

import os
import numpy as np

import concourse.bass as bass
import concourse.tile as tile
from concourse import mybir, bacc, bass_utils
from concourse.masks import make_identity

P = 128
H = 128
N_CORES = 8
N_GRAPHS = 4096
GPC = N_GRAPHS // N_CORES      # graphs per core
GB = 4                         # pool groups (of 128 graphs) per core
NQ = 4                         # table quarters (int16 index range)
BN_EPS = 1e-5
NW = 4                         # blocks per gather group / PSUM streams

F32 = mybir.dt.float32
F16 = mybir.dt.float16
BF16 = mybir.dt.bfloat16
I32 = mybir.dt.int32
I16 = mybir.dt.int16

LAST_EXEC_NS = None
_CACHE = {}


def _table_row_fn(NPC, NPQ, QRNG, seg):
    """Table layout: [quarter][segment][core][row]. seg AllGather segments
    per quarter keep each segment's AllGather output rank-contiguous."""
    NPS = NPQ // seg
    SRNG = N_CORES * NPS

    def tr(gid):
        c = gid // NPC
        i = gid % NPC
        iq = i % NPQ
        return (i // NPQ) * QRNG + (iq // NPS) * SRNG + c * NPS + (iq % NPS)
    return tr


def _preprocess(x, src, dst, batch, dinv, fill_min=0.0, agseg=1):
    """Host-side sharding: node remap + per-core padded meta arrays.

    fill_min > 0 enables "direct" chunks: per (group, quarter, block), round k
    takes at most one edge per dst slot (slot == partition == dst), while the
    mean fill over cores stays >= fill_min. Direct chunks need no one-hot --
    the device accumulates them with a constant-identity matmul; holes point
    at a guaranteed-zero table row (a padding row of the quarter).
    Leftover edges go to the usual one-hot pair chunks.
    """
    N = x.shape[0]
    graph_start = np.searchsorted(batch, np.arange(N_GRAPHS + 1))
    seg_rows = np.zeros((N_CORES, GB), dtype=np.int64)
    for c in range(N_CORES):
        for g in range(GB):
            g0 = c * GPC + g * P
            seg_rows[c, g] = graph_start[g0 + P] - graph_start[g0]
    C2 = int(np.ceil(seg_rows.max() / P))     # node blocks per pool group
    NBLK = GB * C2                            # node blocks per core
    NPC = NBLK * P                            # padded nodes per core
    NGRP = (NBLK + NW - 1) // NW

    newid = np.zeros(N, dtype=np.int64)
    for c in range(N_CORES):
        for g in range(GB):
            g0 = c * GPC + g * P
            r0, r1 = graph_start[g0], graph_start[g0 + P]
            newid[r0:r1] = c * NPC + g * C2 * P + np.arange(r1 - r0)

    xT_loc = np.zeros((N_CORES, H, NPC), dtype=np.float32)
    dinvb = np.zeros((N_CORES, P, NBLK), dtype=np.float32)   # 0 for padding
    glocb = np.full((N_CORES, P, NBLK), -1.0, dtype=np.float32)
    invcnt = np.ones((N_CORES, P, GB), dtype=np.float32)
    loc_all = newid % NPC
    core_all = newid // NPC
    for c in range(N_CORES):
        m = core_all == c
        loc = loc_all[m]
        xT_loc[c][:, loc] = x[m].T
        dinvb[c, loc % P, loc // P] = dinv[m]
        gl = (batch[m] - c * GPC).astype(np.int64)      # 0..GPC-1
        glocb[c, loc % P, loc // P] = (gl % P).astype(np.float32)
        cnt = np.zeros(GPC, dtype=np.float64)
        np.add.at(cnt, gl, 1.0)
        invcnt[c] = (1.0 / np.maximum(cnt, 1.0)).reshape(GB, P).T.astype(np.float32)

    # per-column dinv rows for the post-sum scale: [P, NBLK*P], every
    # partition holds the same row; padding columns are 0.
    dinvrow = np.zeros((N_CORES, P, NBLK * P), dtype=np.float32)
    for c in range(N_CORES):
        row = dinvb[c].T.reshape(-1)          # [NBLK*P]: col b*P+s = dinv(b,s)
        dinvrow[c] = np.broadcast_to(row, (P, NBLK * P))

    # edges grouped by (4-block group, src quarter, dst block); self-loops
    # handled by plain-identity matmuls on device. table rows live in
    # [quarter][core][row] order (quarter AllGathers).
    NPQ = NPC // GB
    QRNG = N_CORES * NPQ                      # rows per table quarter
    table_row = _table_row_fn(NPC, NPQ, QRNG, agseg)
    e_src_g = table_row(newid[src])
    e_q = e_src_g // QRNG
    e_ridx = (e_src_g % QRNG).astype(np.int16)
    e_dst_core = core_all[dst]
    e_dst_loc = loc_all[dst]

    # chunks span block boundaries within a (group, quarter): edges are
    # packed densely per (gg, q) sorted by block; each (chunk, block) overlap
    # becomes a one-hot "pair" column that masks the other blocks' slots.
    NK2 = NGRP * NQ
    e_blk = e_dst_loc // P
    e_gq = (e_blk // NW) * NQ + e_q
    e_slot = e_dst_loc % P

    # zero (padding) table row per quarter, for direct-chunk holes
    zero_ridx = np.full(NQ, -1, dtype=np.int64)
    for q in range(NQ):
        c_star = int(np.argmin(seg_rows[:, q]))
        if seg_rows[c_star, q] < C2 * P:
            gid0 = c_star * NPC + q * NPQ + seg_rows[c_star, q]
            zero_ridx[q] = table_row(np.int64(gid0)) % QRNG

    # per-core edges sorted by (gq, blk, slot), with per-slot rank
    edata = []
    KMAX = 8
    nslots_ge = np.zeros((N_CORES, NK2, NBLK, KMAX), dtype=np.int64)
    for c in range(N_CORES):
        m = e_dst_core == c
        key2 = (e_gq[m] * NBLK + e_blk[m]) * P + e_slot[m]
        order = np.argsort(key2, kind="stable")
        gq = e_gq[m][order]
        blk = e_blk[m][order]
        slot = e_slot[m][order]
        ridx = e_ridx[m][order]
        key2s = key2[order]
        first = np.concatenate([[0], np.flatnonzero(np.diff(key2s)) + 1])
        starts = np.zeros(len(key2s), dtype=np.int64)
        starts[first] = first
        starts = np.maximum.accumulate(starts)
        rank = np.arange(len(key2s)) - starts
        cnt = np.bincount(key2s, minlength=NK2 * NBLK * P).reshape(
            NK2, NBLK, P)
        for k in range(KMAX):
            nslots_ge[c, :, :, k] = (cnt > k).sum(axis=2)
        edata.append((gq, blk, slot, ridx, rank))

    # shared direct-round counts per (gq, b): mean fill over cores >= fill_min
    K_dir = np.zeros((NK2, NBLK), dtype=np.int64)
    if fill_min > 0:
        mean_fill = nslots_ge.mean(axis=0) / P          # [NK2, NBLK, KMAX]
        for k in range(KMAX):
            K_dir[(mean_fill[:, :, k] >= fill_min) & (K_dir == k)] = k + 1
        # quarters without a zero row can't host direct holes
        for q in range(NQ):
            if zero_ridx[q] < 0:
                K_dir[q::NQ, :] = 0

    # leftover (pair) edges per core/stream; shared pair chunk counts
    n_gq = np.zeros((N_CORES, NK2), dtype=np.int64)
    for c in range(N_CORES):
        gq, blk, slot, ridx, rank = edata[c]
        left = rank >= K_dir[gq, blk]
        n_gq[c] = np.bincount(gq[left], minlength=NK2)
    NCH_gq = -(-n_gq.max(axis=0) // P)                   # pair chunks
    ND_gq = np.array([[K_dir[gg * NQ + q,
                             gg * NW:min((gg + 1) * NW, NBLK)].sum()
                       for q in range(NQ)] for gg in range(NGRP)]
                     ).reshape(-1)                        # direct chunks
    J_gq = ND_gq + NCH_gq                                # total chunks
    chunkbase = np.concatenate([[0], np.cumsum(J_gq)])
    NCHT = int(chunkbase[-1])                 # total chunk columns (idx/g)

    # union pair list (gq, pair chunk j, block): encoded, sorted => (gq,j,b)
    JMAX = int(NCH_gq.max()) if NCH_gq.max() > 0 else 1
    encs = []
    pairpos = []                               # per-core leftover positions
    for c in range(N_CORES):
        gq, blk, slot, ridx, rank = edata[c]
        left = np.flatnonzero(rank >= K_dir[gq, blk])
        lgq = gq[left]
        start2 = np.concatenate(
            [[0], np.cumsum(np.bincount(lgq, minlength=NK2))])
        pos = np.arange(len(left)) - start2[lgq]
        pairpos.append((left, pos))
        encs.append((lgq * JMAX + pos // P) * NBLK + blk[left])
    union = np.unique(np.concatenate(encs))
    NPAIR = max(len(union), 1)
    pair_gq = union // (JMAX * NBLK)
    pair_j = (union // NBLK) % JMAX
    pair_b = union % NBLK
    np_gq = np.bincount(pair_gq, minlength=NK2)
    pairbase = np.concatenate([[0], np.cumsum(np_gq)])

    dstl = np.full((N_CORES, P, NPAIR), -1.0, dtype=np.float32)
    idx16 = np.zeros((N_CORES, P, 8 * NCHT), dtype=np.int16)
    pp_ = np.arange(P)
    for c in range(N_CORES):
        gq, blk, slot, ridx, rank = edata[c]
        wrapped = np.zeros((16, 8 * NCHT), dtype=np.int16)
        # direct chunks: idx position == dst slot; holes -> zero row
        if fill_min > 0:
            # chunk column of direct round k of (gq, b):
            # chunkbase[gq] + sum(K_dir[gq, blocks<b]) + k
            kd_cum = np.concatenate(
                [np.zeros((NK2, 1), dtype=np.int64),
                 np.cumsum(K_dir, axis=1)], axis=1)
            gg_first = (np.arange(NK2) // NQ) * NW
            dmask = rank < K_dir[gq, blk]
            dcol = (chunkbase[gq] + kd_cum[gq, blk]
                    - kd_cum[gq, gg_first[gq]] + rank)[dmask]
            dslot = slot[dmask]
            dridx = ridx[dmask]
            # initialize all direct cells to the quarter's zero row
            for k2 in range(NK2):
                q = k2 % NQ
                if zero_ridx[q] < 0:
                    continue
                c0 = chunkbase[k2]
                nd = ND_gq[k2]
                if nd == 0:
                    continue
                cols = np.arange(c0, c0 + nd)
                wrapped[:, (8 * cols[:, None] +
                            np.arange(8)[None, :]).ravel()] = np.int16(
                    zero_ridx[q])
            wrapped[dslot % 16, 8 * dcol + dslot // 16] = dridx
        # pair chunks
        left, pos = pairpos[c]
        j = pos // P
        p = pos % P
        lgq = gq[left]
        paircol = np.searchsorted(
            union, (lgq * JMAX + j) * NBLK + blk[left])
        dstl[c, p, paircol] = slot[left].astype(np.float32)
        chunkcol = chunkbase[lgq] + ND_gq[lgq] + j
        wrapped[p % 16, 8 * chunkcol + p // 16] = ridx[left]
        # trailing pair-chunk padding -> -1 (ucode trims trailing negatives:
        # no descriptors generated, no DMA bytes moved). NOTE: hangs the
        # SWDGE ring bookkeeping on HW (decode sizes the ring from the
        # untrimmed count) -- keep off unless BASS_GCN_TRIM=1.
        for k2 in (range(NK2) if os.environ.get("BASS_GCN_TRIM", "0") == "1"
                   else ()):
            n_real = int(n_gq[c, k2])
            n_slots = int(NCH_gq[k2]) * P
            if n_real >= n_slots:
                continue
            pos = np.arange(n_real, n_slots)
            pslot = pos % P
            ccol = chunkbase[k2] + ND_gq[k2] + pos // P
            wrapped[pslot % 16, 8 * ccol + pslot // 16] = -1
        idx16[c] = wrapped[pp_ % 16, :]

    # per-(gg,q) build tables
    J_t = J_gq.reshape(NGRP, NQ)
    chunkb_t = chunkbase[:-1].reshape(NGRP, NQ)
    pairs_t = []
    dirs_t = []
    for gg in range(NGRP):
        prow = []
        drow = []
        for q in range(NQ):
            k = gg * NQ + q
            sel = slice(int(pairbase[k]), int(pairbase[k + 1]))
            prow.append(tuple(zip(pair_j[sel].tolist(),
                                  pair_b[sel].tolist())))
            dd = []
            for b in range(gg * NW, min((gg + 1) * NW, NBLK)):
                dd.extend([b] * int(K_dir[k, b]))
            drow.append(tuple(dd))
        pairs_t.append(tuple(prow))
        dirs_t.append(tuple(drow))
    pairs_t = tuple(pairs_t)
    dirs_t = tuple(dirs_t)
    pairb_t = pairbase[:-1].reshape(NGRP, NQ)

    return dict(C2=C2, NBLK=NBLK, NPC=NPC, NCHT=NCHT, NPAIR=NPAIR,
                NGRP=NGRP, NCH_t=J_t, chunkb_t=chunkb_t,
                pairs_t=pairs_t, dirs_t=dirs_t, pairb_t=pairb_t,
                xT_loc=xT_loc, dinvb=dinvb, glocb=glocb, invcnt=invcnt,
                dinvrow=dinvrow, idx16=idx16, dstl=dstl)


def _build(C2, NBLK, NPC, NCHT, NPAIR, NGRP, NCH_t, chunkb_t, pairs_t,
           dirs_t, pairb_t, hb2_val, queues=(0, 1), jcap=8,
           shared_tfull=True, single_packet=True, agseg=1, debug=False):
    JMAXQ = int(NCH_t.max())              # g buffer: chunks per (gg, q)
    JGMAX = int(NCH_t.sum(axis=1).max())  # idx staging: chunks per group
    PMAXQ = max(len(pr) for row in pairs_t for pr in row)  # oh buffer: pairs
    JCAP = jcap  # chunks per dma_gather instr (SWDGE ring capacity bound)
    table_dt = F16
    nc = bacc.Bacc("TRN2", target_bir_lowering=False, debug=False,
                   num_devices=N_CORES, num_swdge_queues=4)
    xT_d = nc.dram_tensor("xT_loc", [H, NPC], BF16, kind="ExternalInput")
    idx16_d = nc.dram_tensor("idx16", [P, 8 * NCHT], I16,
                             kind="ExternalInput")
    dstl_d = nc.dram_tensor("dstl", [P, NPAIR], table_dt,
                            kind="ExternalInput")
    dinvb_d = nc.dram_tensor("dinvb", [P, NBLK], F32, kind="ExternalInput")
    dinvrow_d = nc.dram_tensor("dinvrow", [P, NBLK * P], F16,
                               kind="ExternalInput")
    glocb_d = nc.dram_tensor("glocb", [P, NBLK], F32, kind="ExternalInput")
    invcnt_d = nc.dram_tensor("invcnt", [P, GB], F32, kind="ExternalInput")
    W_d = nc.dram_tensor("Wsb", [H, 4 * H], BF16, kind="ExternalInput")
    scol_d = nc.dram_tensor("scol", [P, 4], F32, kind="ExternalInput")
    sbcol_d = nc.dram_tensor("sbcol", [P, 4], F32, kind="ExternalInput")
    b2col_d = nc.dram_tensor("b2col", [P, 4], F32, kind="ExternalInput")
    srep3_d = nc.dram_tensor("srep3", [P, H], F32, kind="ExternalInput")
    sbrep3_d = nc.dram_tensor("sbrep3", [P, H], F32, kind="ExternalInput")
    b2rep3_d = nc.dram_tensor("b2rep3", [P, H], F32, kind="ExternalInput")
    iota16_d = nc.dram_tensor("iota16", [P, P], table_dt, kind="ExternalInput")
    iota32_d = nc.dram_tensor("iota32", [P, P], F32, kind="ExternalInput")
    hW1_d = nc.dram_tensor("hW1", [H, H], F32, kind="ExternalInput")
    hb1rep_d = nc.dram_tensor("hb1rep", [P, H], F32, kind="ExternalInput")
    hW2_d = nc.dram_tensor("hW2", [H, 1], F32, kind="ExternalInput")
    out_d = nc.dram_tensor("out", [GPC, 1], F32, kind="ExternalOutput")
    hd_d = [nc.dram_tensor(f"hdump{l}", [P, NBLK * H], F32,
                           kind="ExternalOutput")
            for l in range(4)] if debug else None
    td_d = (nc.dram_tensor("tdump", [P, NBLK * H], F32,
                           kind="ExternalOutput") if debug else None)

    NPQ = NPC // GB
    QRNG = N_CORES * NPQ
    t_loc = [[nc.dram_tensor(f"t_loc{l}_{q}", [NPQ, H], table_dt)
              for q in range(GB)] for l in range(4)]
    tf_kw = {"addr_space": "Shared"} if shared_tfull else {}
    T_full = [nc.dram_tensor(f"T_full{l}", [N_CORES * NPC, H], table_dt,
                             **tf_kw)
              for l in range(4)]

    with tile.TileContext(nc) as tc:
        with (
            tc.tile_pool(name="persist", bufs=1) as pp,
            tc.tile_pool(name="stagea", bufs=3) as sap,
            tc.tile_pool(name="streamg", bufs=3) as spg,
            tc.tile_pool(name="streamo", bufs=2) as spo,
            tc.tile_pool(name="idxs", bufs=3) as spi,
            tc.tile_pool(name="pool2", bufs=1) as wp2,
            tc.tile_pool(name="psum_agg", bufs=1, space="PSUM") as psagg_tp,
            tc.tile_pool(name="psum_a", bufs=2, space="PSUM") as psa_tp,
            tc.tile_pool(name="psum_p", bufs=1, space="PSUM") as psp_tp,
        ):
            h_sb = pp.tile([P, NBLK * H], BF16)
            t_sb = pp.tile([P, NBLK * H], table_dt)
            dstl = pp.tile([P, NPAIR], table_dt)
            dinvb = pp.tile([P, NBLK], F32)
            glocb = pp.tile([P, NBLK], F32)
            invcnt = pp.tile([P, GB], F32)
            W_sb = pp.tile([H, 4 * H], BF16)
            scol = pp.tile([P, 4], F32)
            sbcol = pp.tile([P, 4], F32)
            b2col = pp.tile([P, 4], F32)
            srep3 = pp.tile([P, H], F32)
            sbrep3 = pp.tile([P, H], F32)
            b2rep3 = pp.tile([P, H], F32)
            iota16 = pp.tile([P, P], table_dt)
            iota32 = pp.tile([P, P], F32)
            hW1_sb = pp.tile([H, H], F32)
            hb1rep = pp.tile([P, H], F32)
            hW2_sb = pp.tile([H, 1], F32)
            ident = pp.tile([P, P], F32)
            ident16 = pp.tile([P, P], table_dt)
            z2all = pp.tile([1, GPC], F32)
            for sb, d in [(dstl, dstl_d),
                          (dinvb, dinvb_d),
                          (glocb, glocb_d),
                          (invcnt, invcnt_d), (W_sb, W_d),
                          (scol, scol_d), (sbcol, sbcol_d), (b2col, b2col_d),
                          (srep3, srep3_d), (sbrep3, sbrep3_d),
                          (b2rep3, b2rep3_d),
                          (iota16, iota16_d), (iota32, iota32_d),
                          (hW1_sb, hW1_d), (hb1rep, hb1rep_d),
                          (hW2_sb, hW2_d)]:
                nc.sync.dma_start(sb[:], d[:])
            make_identity(nc, ident[:])
            nc.vector.tensor_copy(ident16[:], ident[:])
            # per-quarter x load so quarter-0 t-builds start early
            NPQ_ = NPC // GB
            for q in range(GB):
                nc.sync.dma_start(h_sb[:, q * NPQ_:(q + 1) * NPQ_],
                                  xT_d[:, q * NPQ_:(q + 1) * NPQ_])

            # zero-fill both buffer instances of each gather tile so slots
            # skipped by trailing-negative idx trimming read finite data
            for q in range(NQ):
                for _ in range(3):
                    gz = spg.tile([P, JMAXQ * H], table_dt, name=f"g{q}")
                    nc.vector.memset(gz[:], 0.0)

            ps_st = [psagg_tp.tile([P, P], F32, space="PSUM", name=f"psagg{s}")
                     for s in range(NW)]

            # hoisted num_idxs registers (one MOVE per distinct value)
            nidx_regs = {}

            def nidx_reg(v):
                if v not in nidx_regs:
                    nidx_regs[v] = nc.gpsimd.to_reg(v)
                return nidx_regs[v]

            # round-robin gather queue assignment
            qstate = [0]

            def next_queue():
                q = queues[qstate[0] % len(queues)]
                qstate[0] += 1
                return q

            def emit_gather_parts(gg, T_l):
                """Per-quarter gathers (split to fit the SWDGE descriptor
                ring) + one one-hot build per quarter, for a block group."""
                parts = []
                cg0 = int(chunkb_t[gg, 0])
                jg = int(NCH_t[gg].sum())
                idxg = spi.tile([P, 8 * JGMAX], I16, name="idxg")
                nc.sync.dma_start(idxg[:, 0:8 * jg],
                                  idx16_d[:, 8 * cg0:8 * (cg0 + jg)])
                for q in range(NQ):
                    J = int(NCH_t[gg, q])
                    NP = len(pairs_t[gg][q])
                    if J == 0:
                        continue
                    c0 = int(chunkb_t[gg, q])
                    p0 = int(pairb_t[gg, q])
                    g = spg.tile([P, JMAXQ * H], table_dt, name=f"g{q}")
                    oh = (spo.tile([P, PMAXQ * P], table_dt, name=f"oh{q}")
                          if NP > 0 else None)
                    gap = g[:]
                    # balanced split: ceil(J/JCAP) near-equal pieces
                    nsplit = -(-J // JCAP)
                    done = 0
                    for i in range(nsplit):
                        Jp = J // nsplit + (1 if i < J % nsplit else 0)
                        cc = c0 + done
                        out3 = bass.AP(gap.tensor,
                                       gap.offset + done * H,
                                       [gap.ap[0], [H, Jp], [1, H]])
                        nc.gpsimd.dma_gather(
                            out_ap=out3,
                            in_ap=T_l[q * QRNG:(q + 1) * QRNG, :],
                            idxs_ap=idxg[:, 8 * (cc - cg0):
                                         8 * (cc - cg0 + Jp)],
                            num_idxs=P * Jp,
                            num_idxs_reg=nidx_reg(P * Jp),
                            elem_size=H,
                            single_packet=single_packet,
                            queue_num=next_queue(),
                        )
                        done += Jp
                    if NP > 0:
                        oh_ap = oh[:]
                        oh3 = bass.AP(oh_ap.tensor, oh_ap.offset,
                                      [oh_ap.ap[0], [P, NP], [1, P]])
                        ia = iota16[:]
                        iota3 = bass.AP(ia.tensor, ia.offset,
                                        [ia.ap[0], [0, NP], ia.ap[1]])
                        nc.vector.tensor_tensor(
                            out=oh3,
                            in0=dstl[:, p0:p0 + NP].to_broadcast([P, NP, P]),
                            in1=iota3, op=mybir.AluOpType.is_equal)
                    parts.append((q, g, oh))
                return parts

            def emit_t_block(l, b):
                # t_l[block b] = dinv * (hT[block b]^T @ W_l), into t_loc[l]
                # hT block is [h, node]; lhsT = hT -> out [node, h'].
                ls_t = slice(l * H, (l + 1) * H)
                tps = psa_tp.tile([P, H], F32, space="PSUM", name="tps")
                nc.tensor.matmul(tps[:], lhsT=h_sb[:, b * H:(b + 1) * H],
                                 rhs=W_sb[:, ls_t],
                                 start=True, stop=True, skip_group_check=True)
                nc.scalar.activation(t_sb[:, b * H:(b + 1) * H], tps[:],
                                     mybir.ActivationFunctionType.Copy,
                                     scale=dinvb[:, b:b + 1])
                q, bq = divmod(b, NBLK // GB)
                nc.sync.dma_start(t_loc[l][q][bq * P:(bq + 1) * P, :],
                                  t_sb[:, b * H:(b + 1) * H])
                if debug and l == 0:
                    tf = sap.tile([P, H], F32, name="tdmp")
                    nc.vector.tensor_copy(tf[:], t_sb[:, b * H:(b + 1) * H])
                    nc.sync.dma_start(td_d[:, b * H:(b + 1) * H], tf[:])

            C2b = NBLK // GB   # blocks per pool quarter
            NPS = NPQ // agseg
            SRNG = N_CORES * NPS
            BPS = C2b // agseg  # blocks per AllGather segment
            NSEG = GB * agseg

            def emit_ag(l, seg):
                q, s = divmod(seg, agseg)
                base = q * QRNG + s * SRNG
                nc.gpsimd.collective_compute(
                    "AllGather", mybir.AluOpType.bypass,
                    replica_groups=[list(range(N_CORES))],
                    ins=[t_loc[l][q][s * NPS:(s + 1) * NPS, :]],
                    outs=[T_full[l][base:base + SRNG, :]])

            with nc.named_scope("stageA0"):
                nq_ = 0
                for b in range(NBLK):
                    emit_t_block(0, b)
                    while nq_ < NSEG and b >= (nq_ + 1) * BPS - 1:
                        emit_ag(0, nq_)
                        nq_ += 1

            def emit_head(gb, pps):
                pooled = wp2.tile([P, H], F32, name="pooled")
                nc.vector.tensor_scalar(pooled[:], pps[:],
                                        invcnt[:, gb:gb + 1], None,
                                        mybir.AluOpType.mult)
                # head: relu(pooled @ hW1 + hb1) @ hW2 + hb2
                trp = psp_tp.tile([P, H], F32, space="PSUM", name="pA")
                nc.tensor.transpose(out=trp[:], in_=pooled[:],
                                    identity=ident[:])
                poolT = wp2.tile([P, H], F32, name="poolT")
                nc.scalar.copy(poolT[:], trp[:])
                z1ps = psp_tp.tile([P, H], F32, space="PSUM", name="pA")
                nc.tensor.matmul(z1ps[:], lhsT=poolT[:], rhs=hW1_sb[:],
                                 start=True, stop=True,
                                 skip_group_check=True)
                r1 = wp2.tile([P, H], F32, name="r1")
                nc.vector.tensor_tensor(out=r1[:], in0=z1ps[:],
                                        in1=hb1rep[:],
                                        op=mybir.AluOpType.add)
                nc.scalar.activation(r1[:], r1[:],
                                     mybir.ActivationFunctionType.Relu)
                tr2 = psp_tp.tile([P, H], F32, space="PSUM", name="pA")
                nc.tensor.transpose(out=tr2[:], in_=r1[:], identity=ident[:])
                r1T = wp2.tile([P, H], F32, name="r1T")
                nc.scalar.copy(r1T[:], tr2[:])
                z2full = psp_tp.tile([P, P], F32, space="PSUM", name="pA")
                z2ps = z2full[0:1, :]
                nc.tensor.matmul(z2ps[:], lhsT=hW2_sb[:], rhs=r1T[:],
                                 start=True, stop=True,
                                 skip_group_check=True)
                nc.vector.tensor_scalar(
                    z2all[0:1, gb * P:(gb + 1) * P], z2ps[:],
                    float(hb2_val), None, mybir.AluOpType.add)

            for l in range(4):
                flip = l < 3
                with nc.named_scope(f"agg{l}"):
                    nq_ = 0
                    for gg in range(NGRP):
                        blocks = list(range(gg * NW, min((gg + 1) * NW, NBLK)))
                        if flip:
                            # per-group slice of the post-sum dinv[dst] rows
                            dvr = sap.tile([P, NW * P], F16, name="dvr")
                            nc.sync.dma_start(
                                dvr[:, 0:len(blocks) * P],
                                dinvrow_d[:, gg * NW * P:
                                          (gg * NW + len(blocks)) * P])
                        parts = emit_gather_parts(gg, T_full[l])
                        rem = {b: sum(1 for q in range(NQ)
                                      for (_, bb) in pairs_t[gg][q]
                                      if bb == b)
                               + sum(1 for q in range(NQ)
                                     for bb in dirs_t[gg][q]
                                     if bb == b)
                               for b in blocks}
                        for st, b in enumerate(blocks):
                            # self-loop: plain identity (table rows already
                            # carry dinv[src]; dinv[dst] applied post-sum)
                            tblk = t_sb[:, b * H:(b + 1) * H]
                            ps = ps_st[st]
                            if flip:
                                nc.tensor.matmul(ps[:], lhsT=tblk,
                                                 rhs=ident16[:], start=True,
                                                 stop=(rem[b] == 0),
                                                 skip_group_check=True)
                            else:
                                nc.tensor.matmul(ps[:], lhsT=ident16[:],
                                                 rhs=tblk, start=True,
                                                 stop=(rem[b] == 0),
                                                 skip_group_check=True)
                        for (q, g, oh) in parts:
                            ndir = len(dirs_t[gg][q])
                            for jd, b in enumerate(dirs_t[gg][q]):
                                st = b - gg * NW
                                ps = ps_st[st]
                                rem[b] -= 1
                                if flip:
                                    nc.tensor.matmul(
                                        ps[:], lhsT=g[:, jd * H:(jd + 1) * H],
                                        rhs=ident16[:],
                                        start=False, stop=(rem[b] == 0),
                                        skip_group_check=True)
                                else:
                                    nc.tensor.matmul(
                                        ps[:], lhsT=ident16[:],
                                        rhs=g[:, jd * H:(jd + 1) * H],
                                        start=False, stop=(rem[b] == 0),
                                        skip_group_check=True)
                            for k, (j, b) in enumerate(pairs_t[gg][q]):
                                st = b - gg * NW
                                ps = ps_st[st]
                                rem[b] -= 1
                                jj = ndir + j
                                if flip:
                                    nc.tensor.matmul(
                                        ps[:], lhsT=g[:, jj * H:(jj + 1) * H],
                                        rhs=oh[:, k * P:(k + 1) * P],
                                        start=False, stop=(rem[b] == 0),
                                        skip_group_check=True)
                                else:
                                    nc.tensor.matmul(
                                        ps[:], lhsT=oh[:, k * P:(k + 1) * P],
                                        rhs=g[:, jj * H:(jj + 1) * H],
                                        start=False, stop=(rem[b] == 0),
                                        skip_group_check=True)
                        for st, b in enumerate(blocks):
                            ps = ps_st[st]
                            if flip:
                                # X = S * dinv[dst] (per-column), then
                                # h = relu(s*X + s*b) + b2, per-partition
                                xs = wp2.tile([P, H], F32, name=f"xs{st}")
                                nc.vector.tensor_tensor(
                                    out=xs[:], in0=ps[:],
                                    in1=dvr[:, st * P:(st + 1) * P],
                                    op=mybir.AluOpType.mult)
                                nc.scalar.activation(
                                    h_sb[:, b * H:(b + 1) * H], xs[:],
                                    mybir.ActivationFunctionType.Relu,
                                    bias=sbcol[:, l:l + 1],
                                    scale=scol[:, l:l + 1])
                                nc.scalar.activation(
                                    h_sb[:, b * H:(b + 1) * H],
                                    h_sb[:, b * H:(b + 1) * H],
                                    mybir.ActivationFunctionType.Identity,
                                    bias=b2col[:, l:l + 1])
                                if debug:
                                    hf = sap.tile([P, H], F32, name="hdmp")
                                    nc.vector.tensor_copy(
                                        hf[:], h_sb[:, b * H:(b + 1) * H])
                                    nc.sync.dma_start(
                                        hd_d[l][:, b * H:(b + 1) * H], hf[:])
                                emit_t_block(l + 1, b)
                            else:
                                # e = dinv[dst]_p * S, then BN epilogue with
                                # per-channel (column) constants
                                e0 = wp2.tile([P, H], F32, name=f"e0_{st}")
                                e1 = wp2.tile([P, H], F32, name=f"e1_{st}")
                                nc.vector.tensor_scalar(
                                    e0[:], ps[:], dinvb[:, b:b + 1], None,
                                    mybir.AluOpType.mult)
                                nc.vector.tensor_tensor(
                                    out=e1[:], in0=e0[:], in1=srep3[:],
                                    op=mybir.AluOpType.mult)
                                nc.vector.tensor_tensor(
                                    out=e0[:], in0=e1[:], in1=sbrep3[:],
                                    op=mybir.AluOpType.add)
                                nc.scalar.activation(
                                    e1[:], e0[:],
                                    mybir.ActivationFunctionType.Relu)
                                nc.vector.tensor_tensor(
                                    out=h_sb[:, b * H:(b + 1) * H],
                                    in0=e1[:], in1=b2rep3[:],
                                    op=mybir.AluOpType.add)
                                if debug:
                                    hf = sap.tile([P, H], F32, name="hdmp")
                                    nc.vector.tensor_copy(
                                        hf[:], h_sb[:, b * H:(b + 1) * H])
                                    nc.sync.dma_start(
                                        hd_d[l][:, b * H:(b + 1) * H], hf[:])
                                # fused global-mean-pool accumulation
                                gb, kk = divmod(b, C2)
                                ohp = wp2.tile([P, P], BF16, name="ohp")
                                nc.vector.tensor_tensor(
                                    out=ohp[:],
                                    in0=glocb[:, b:b + 1].to_broadcast([P, P]),
                                    in1=iota32[:],
                                    op=mybir.AluOpType.is_equal)
                                pps = psp_tp.tile([P, H], F32, space="PSUM",
                                                  name="pps")
                                nc.tensor.matmul(
                                    pps[:], lhsT=ohp[:],
                                    rhs=h_sb[:, b * H:(b + 1) * H],
                                    start=(kk == 0), stop=(kk == C2 - 1),
                                    skip_group_check=True)
                                if kk == C2 - 1:
                                    emit_head(gb, pps)
                        if flip:
                            last_b = blocks[-1]
                            while nq_ < NSEG and last_b >= (nq_ + 1) * BPS - 1:
                                emit_ag(l + 1, nq_)
                                nq_ += 1

            with nc.named_scope("pool"):
                nc.sync.dma_start(out_d[:, 0:1], z2all[0:1, :])

    nc.compile()
    return nc


def kernel(**inputs):
    global LAST_EXEC_NS
    x = np.ascontiguousarray(np.asarray(inputs["x"], dtype=np.float32))
    ei = np.asarray(inputs["edge_index"]).astype(np.int64)
    batch = np.asarray(inputs["batch"]).astype(np.int64)
    Ws = np.asarray(inputs["Ws"], dtype=np.float32)
    bs = np.asarray(inputs["bs"], dtype=np.float32)
    gammas = np.asarray(inputs["gammas"], dtype=np.float32)
    betas = np.asarray(inputs["betas"], dtype=np.float32)
    bn_means = np.asarray(inputs["bn_means"], dtype=np.float32)
    bn_vars = np.asarray(inputs["bn_vars"], dtype=np.float32)
    hW1 = np.asarray(inputs["hW1"], dtype=np.float32)
    hb1 = np.asarray(inputs["hb1"], dtype=np.float32)
    hW2 = np.asarray(inputs["hW2"], dtype=np.float32)
    hb2 = np.asarray(inputs["hb2"], dtype=np.float32)

    src, dst = ei[0], ei[1]
    N = x.shape[0]
    deg = np.bincount(dst, minlength=N).astype(np.float64) + 1.0
    dinv = (1.0 / np.sqrt(deg)).astype(np.float32)

    fill_min = float(os.environ.get("BASS_GCN_FILL", "0"))
    agseg = int(os.environ.get("BASS_GCN_AGSEG", "1"))
    # segment split requires whole blocks per segment
    NPQ_chk = None  # C2 known only after preprocess; re-checked below
    meta = _preprocess(x, src, dst, batch, dinv, fill_min=fill_min,
                       agseg=agseg)
    if meta["C2"] % agseg != 0:
        agseg = 1
        meta = _preprocess(x, src, dst, batch, dinv, fill_min=fill_min,
                           agseg=1)
    C2, NBLK, NPC, NCHT, NPAIR, NGRP = (meta[k] for k in
                                        ("C2", "NBLK", "NPC", "NCHT",
                                         "NPAIR", "NGRP"))

    debug = os.environ.get("BASS_GCN_DEBUG", "") == "1"
    queues = tuple(int(v) for v in
                   os.environ.get("BASS_GCN_QUEUES", "0,1,2,3").split(","))
    jcap = int(os.environ.get("BASS_GCN_JCAP", "17"))
    shared_tfull = os.environ.get("BASS_GCN_SHARED", "1") == "1"
    single_packet = os.environ.get("BASS_GCN_SP", "0") == "1"
    key = (C2, NBLK, NPC, NCHT, NPAIR, NGRP, meta["pairs_t"],
           meta["dirs_t"],
           tuple(meta["NCH_t"].ravel().tolist()), float(hb2[0]),
           queues, jcap, shared_tfull, single_packet, agseg, debug)
    if key not in _CACHE:
        _CACHE[key] = _build(C2, NBLK, NPC, NCHT, NPAIR, NGRP,
                             meta["NCH_t"], meta["chunkb_t"],
                             meta["pairs_t"], meta["dirs_t"],
                             meta["pairb_t"],
                             float(hb2[0]), queues=queues, jcap=jcap,
                             shared_tfull=shared_tfull,
                             single_packet=single_packet, agseg=agseg,
                             debug=debug)
    nc = _CACHE[key]

    bf16 = mybir.dt.np(BF16)
    # replicated constant arrays
    s_l = gammas / np.sqrt(bn_vars + BN_EPS)            # [4, H]
    b2_l = betas - bn_means * s_l                        # [4, H]
    sb_l = s_l * bs                                      # [4, H]
    Wsb = np.ascontiguousarray(
        Ws.transpose(1, 0, 2).reshape(H, 4 * H)).astype(bf16)
    scol = np.ascontiguousarray(s_l.T)                   # [H, 4]
    sbcol = np.ascontiguousarray(sb_l.T)
    b2col = np.ascontiguousarray(b2_l.T)
    srep3 = np.broadcast_to(s_l[3][None, :], (P, H)).copy()
    sbrep3 = np.broadcast_to(sb_l[3][None, :], (P, H)).copy()
    b2rep3 = np.broadcast_to(b2_l[3][None, :], (P, H)).copy()
    iota16 = np.broadcast_to(np.arange(P, dtype=np.float16)[None, :],
                             (P, P)).copy()
    iota32 = iota16.astype(np.float32)
    hb1rep = np.broadcast_to(hb1[None, :], (P, H)).copy()

    in_maps = []
    for c in range(N_CORES):
        in_maps.append({
            "xT_loc": meta["xT_loc"][c].astype(bf16),
            "idx16": meta["idx16"][c],
            "dstl": meta["dstl"][c].astype(np.float16),
            "dinvb": meta["dinvb"][c],
            "dinvrow": meta["dinvrow"][c].astype(np.float16),
            "glocb": meta["glocb"][c],
            "invcnt": meta["invcnt"][c],
            "Wsb": Wsb, "scol": scol, "sbcol": sbcol, "b2col": b2col,
            "srep3": srep3, "sbrep3": sbrep3, "b2rep3": b2rep3,
            "iota16": iota16, "iota32": iota32,
            "hW1": hW1, "hb1rep": hb1rep, "hW2": hW2,
        })

    trace = os.environ.get("BASS_GCN_TRACE", "") == "1"
    if trace:
        bass_utils.upload_artifacts = lambda tmpdir: "local://" + tmpdir
        try:
            import sys, types
            if "antenv.axon_hooks" not in sys.modules:
                mod = types.ModuleType("antenv.axon_hooks")
                _h = [None]
                mod.set_axon_ntff_profile_hook = lambda h: _h.__setitem__(0, h)
                mod.get_axon_ntff_profile_hook = lambda: _h[0]
                sys.modules["antenv.axon_hooks"] = mod
                import antenv
                antenv.axon_hooks = mod
                from trn_agent_boot.trn_boot import _ntff_profile_via_ctypes
                mod.set_axon_ntff_profile_hook(
                    _ntff_profile_via_ctypes("/opt/axon/libaxon_pjrt.so"))
        except Exception as e:
            print(f"NTFF hook registration failed: {e}")
    res = bass_utils.run_bass_kernel_spmd(nc, in_maps, list(range(N_CORES)),
                                          trace=trace)
    LAST_EXEC_NS = res.exec_time_ns
    if res.exec_time_ns is not None:
        print(f"HW exec time: {res.exec_time_ns} ns")

    if debug:
        kernel.DEBUG_RES = res.results
        kernel.DEBUG_META = meta
    out = np.concatenate([res.results[c]["out"] for c in range(N_CORES)],
                         axis=0).astype(np.float32)
    return out


# revision 41
# speedup vs baseline: 1.0139x; 1.0139x over previous
"""GCN (4x GCNConv + eval BN + ReLU, global mean pool, 2-layer MLP head) on 8
Trainium2 NeuronCores via Bass/Tile.

Sharding: data-parallel over graphs. 4096 graphs -> 8 cores x 512 contiguous
graphs (batch is sorted). Within a core the 512 graphs form 4 pool groups of
128 graphs; each group's nodes are padded to a multiple of 128 rows so pooling
blocks align with node blocks. Edges live on the core owning their dst node.

Per layer (all on device):
  tt = dinv * (h_local @ W_l)           per-core shard, f16 table
  AllGather tt across the 8 cores       (the only collective)
  S[v]   = sum_{e: dst=v} tt[src_e] + tt[v]      (one-hot scatter matmuls,
                                                  PURE 0/1 one-hots + a plain
                                                  identity for the self loop)
  agg[v] = dinv[v] * S[v]                        (per-column scale, folded
                                                  into the epilogue)
  h = BN_l(relu(agg + b_l))
Key design points vs the original baseline:

* Edge rows fetched with dma_gather (SWDGE) by int16 index, split by table
  QUARTER. Gather queue assignment is round-robin over BASS_GCN_QUEUES
  (default "0,1"): queue q runs on Q7 core pair (2q, 2q+1), and q0/q1 rings
  are measurably faster than q2/q3.
* num_idxs registers are hoisted (one MOVE per distinct value, not one per
  gather instruction).
* The one-hot scatter matrices are built with a SINGLE is_equal op per
  (group, quarter) — the old per-pair dstw MULTIPLY op is gone. The symmetric
  normalization's dst factor dinv[dst] is applied post-sum as a per-column
  multiply of the PSUM block against a host-precomputed dinvrow constant.
  The self-loop term is a plain constant-identity matmul (table rows already
  carry dinv[src]).
* Padding nodes carry dinv=0 in the t-build scale, so their table rows are
  exactly zero in every layer.
* Layers 0-2 run the scatter matmul "flipped" (aggT [h, node] in PSUM);
  layer 3 runs node-major so pooling sees node-major h.
* T_full (the AllGather output) is allocated in the Shared DRAM space --
  HBM-HBM AllGather is faster with Shared outputs.

All data-dependent structure is precomputed host-side into per-core meta
arrays; the chunk layout is maxed over cores so the device program is
identical across cores (SPMD).
"""

import os
import numpy as np

import concourse.bass as bass
import concourse.tile as tile
from concourse import mybir, bacc, bass_utils
from concourse.masks import make_identity

P = 128
H = 128
N_CORES = 8
N_GRAPHS = 4096
GPC = N_GRAPHS // N_CORES      # graphs per core
GB = 4                         # pool groups (of 128 graphs) per core
NQ = 4                         # table quarters (int16 index range)
BN_EPS = 1e-5
NW = 4                         # blocks per gather group / PSUM streams

F32 = mybir.dt.float32
F16 = mybir.dt.float16
BF16 = mybir.dt.bfloat16
I32 = mybir.dt.int32
I16 = mybir.dt.int16

LAST_EXEC_NS = None
_CACHE = {}


def _table_row_fn(NPC, NPQ, QRNG, seg):
    """Table layout: [quarter][segment][core][row]. seg AllGather segments
    per quarter keep each segment's AllGather output rank-contiguous."""
    NPS = NPQ // seg
    SRNG = N_CORES * NPS

    def tr(gid):
        c = gid // NPC
        i = gid % NPC
        iq = i % NPQ
        return (i // NPQ) * QRNG + (iq // NPS) * SRNG + c * NPS + (iq % NPS)
    return tr


def _preprocess(x, src, dst, batch, dinv, fill_min=0.0, agseg=1):
    """Host-side sharding: node remap + per-core padded meta arrays.

    fill_min > 0 enables "direct" chunks: per (group, quarter, block), round k
    takes at most one edge per dst slot (slot == partition == dst), while the
    mean fill over cores stays >= fill_min. Direct chunks need no one-hot --
    the device accumulates them with a constant-identity matmul; holes point
    at a guaranteed-zero table row (a padding row of the quarter).
    Leftover edges go to the usual one-hot pair chunks.
    """
    N = x.shape[0]
    graph_start = np.searchsorted(batch, np.arange(N_GRAPHS + 1))
    seg_rows = np.zeros((N_CORES, GB), dtype=np.int64)
    for c in range(N_CORES):
        for g in range(GB):
            g0 = c * GPC + g * P
            seg_rows[c, g] = graph_start[g0 + P] - graph_start[g0]
    C2 = int(np.ceil(seg_rows.max() / P))     # node blocks per pool group
    NBLK = GB * C2                            # node blocks per core
    NPC = NBLK * P                            # padded nodes per core
    NGRP = (NBLK + NW - 1) // NW

    newid = np.zeros(N, dtype=np.int64)
    for c in range(N_CORES):
        for g in range(GB):
            g0 = c * GPC + g * P
            r0, r1 = graph_start[g0], graph_start[g0 + P]
            newid[r0:r1] = c * NPC + g * C2 * P + np.arange(r1 - r0)

    xT_loc = np.zeros((N_CORES, H, NPC), dtype=np.float32)
    dinvb = np.zeros((N_CORES, P, NBLK), dtype=np.float32)   # 0 for padding
    glocb = np.full((N_CORES, P, NBLK), -1.0, dtype=np.float32)
    invcnt = np.ones((N_CORES, P, GB), dtype=np.float32)
    loc_all = newid % NPC
    core_all = newid // NPC
    for c in range(N_CORES):
        m = core_all == c
        loc = loc_all[m]
        xT_loc[c][:, loc] = x[m].T
        dinvb[c, loc % P, loc // P] = dinv[m]
        gl = (batch[m] - c * GPC).astype(np.int64)      # 0..GPC-1
        glocb[c, loc % P, loc // P] = (gl % P).astype(np.float32)
        cnt = np.zeros(GPC, dtype=np.float64)
        np.add.at(cnt, gl, 1.0)
        invcnt[c] = (1.0 / np.maximum(cnt, 1.0)).reshape(GB, P).T.astype(np.float32)

    # per-column dinv rows for the post-sum scale: [P, NBLK*P], every
    # partition holds the same row; padding columns are 0.
    dinvrow = np.zeros((N_CORES, P, NBLK * P), dtype=np.float32)
    for c in range(N_CORES):
        row = dinvb[c].T.reshape(-1)          # [NBLK*P]: col b*P+s = dinv(b,s)
        dinvrow[c] = np.broadcast_to(row, (P, NBLK * P))

    # edges grouped by (4-block group, src quarter, dst block); self-loops
    # handled by plain-identity matmuls on device. table rows live in
    # [quarter][core][row] order (quarter AllGathers).
    NPQ = NPC // GB
    QRNG = N_CORES * NPQ                      # rows per table quarter
    table_row = _table_row_fn(NPC, NPQ, QRNG, agseg)
    e_src_g = table_row(newid[src])
    e_q = e_src_g // QRNG
    e_ridx = (e_src_g % QRNG).astype(np.int16)
    e_dst_core = core_all[dst]
    e_dst_loc = loc_all[dst]

    # chunks span block boundaries within a (group, quarter): edges are
    # packed densely per (gg, q) sorted by block; each (chunk, block) overlap
    # becomes a one-hot "pair" column that masks the other blocks' slots.
    NK2 = NGRP * NQ
    e_blk = e_dst_loc // P
    e_gq = (e_blk // NW) * NQ + e_q
    e_slot = e_dst_loc % P

    # zero (padding) table row per quarter, for direct-chunk holes
    zero_ridx = np.full(NQ, -1, dtype=np.int64)
    for q in range(NQ):
        c_star = int(np.argmin(seg_rows[:, q]))
        if seg_rows[c_star, q] < C2 * P:
            gid0 = c_star * NPC + q * NPQ + seg_rows[c_star, q]
            zero_ridx[q] = table_row(np.int64(gid0)) % QRNG

    # per-core edges sorted by (gq, blk, slot), with per-slot rank
    edata = []
    KMAX = 8
    nslots_ge = np.zeros((N_CORES, NK2, NBLK, KMAX), dtype=np.int64)
    for c in range(N_CORES):
        m = e_dst_core == c
        key2 = (e_gq[m] * NBLK + e_blk[m]) * P + e_slot[m]
        order = np.argsort(key2, kind="stable")
        gq = e_gq[m][order]
        blk = e_blk[m][order]
        slot = e_slot[m][order]
        ridx = e_ridx[m][order]
        key2s = key2[order]
        first = np.concatenate([[0], np.flatnonzero(np.diff(key2s)) + 1])
        starts = np.zeros(len(key2s), dtype=np.int64)
        starts[first] = first
        starts = np.maximum.accumulate(starts)
        rank = np.arange(len(key2s)) - starts
        cnt = np.bincount(key2s, minlength=NK2 * NBLK * P).reshape(
            NK2, NBLK, P)
        for k in range(KMAX):
            nslots_ge[c, :, :, k] = (cnt > k).sum(axis=2)
        edata.append((gq, blk, slot, ridx, rank))

    # shared direct-round counts per (gq, b): mean fill over cores >= fill_min
    K_dir = np.zeros((NK2, NBLK), dtype=np.int64)
    if fill_min > 0:
        mean_fill = nslots_ge.mean(axis=0) / P          # [NK2, NBLK, KMAX]
        for k in range(KMAX):
            K_dir[(mean_fill[:, :, k] >= fill_min) & (K_dir == k)] = k + 1
        # quarters without a zero row can't host direct holes
        for q in range(NQ):
            if zero_ridx[q] < 0:
                K_dir[q::NQ, :] = 0

    # leftover (pair) edges per core/stream; shared pair chunk counts
    n_gq = np.zeros((N_CORES, NK2), dtype=np.int64)
    for c in range(N_CORES):
        gq, blk, slot, ridx, rank = edata[c]
        left = rank >= K_dir[gq, blk]
        n_gq[c] = np.bincount(gq[left], minlength=NK2)
    NCH_gq = -(-n_gq.max(axis=0) // P)                   # pair chunks
    ND_gq = np.array([[K_dir[gg * NQ + q,
                             gg * NW:min((gg + 1) * NW, NBLK)].sum()
                       for q in range(NQ)] for gg in range(NGRP)]
                     ).reshape(-1)                        # direct chunks
    J_gq = ND_gq + NCH_gq                                # total chunks
    chunkbase = np.concatenate([[0], np.cumsum(J_gq)])
    NCHT = int(chunkbase[-1])                 # total chunk columns (idx/g)

    # union pair list (gq, pair chunk j, block): encoded, sorted => (gq,j,b)
    JMAX = int(NCH_gq.max()) if NCH_gq.max() > 0 else 1
    encs = []
    pairpos = []                               # per-core leftover positions
    for c in range(N_CORES):
        gq, blk, slot, ridx, rank = edata[c]
        left = np.flatnonzero(rank >= K_dir[gq, blk])
        lgq = gq[left]
        start2 = np.concatenate(
            [[0], np.cumsum(np.bincount(lgq, minlength=NK2))])
        pos = np.arange(len(left)) - start2[lgq]
        pairpos.append((left, pos))
        encs.append((lgq * JMAX + pos // P) * NBLK + blk[left])
    union = np.unique(np.concatenate(encs))
    NPAIR = max(len(union), 1)
    pair_gq = union // (JMAX * NBLK)
    pair_j = (union // NBLK) % JMAX
    pair_b = union % NBLK
    np_gq = np.bincount(pair_gq, minlength=NK2)
    pairbase = np.concatenate([[0], np.cumsum(np_gq)])

    dstl = np.full((N_CORES, P, NPAIR), -1.0, dtype=np.float32)
    idx16 = np.zeros((N_CORES, P, 8 * NCHT), dtype=np.int16)
    pp_ = np.arange(P)
    for c in range(N_CORES):
        gq, blk, slot, ridx, rank = edata[c]
        wrapped = np.zeros((16, 8 * NCHT), dtype=np.int16)
        # direct chunks: idx position == dst slot; holes -> zero row
        if fill_min > 0:
            # chunk column of direct round k of (gq, b):
            # chunkbase[gq] + sum(K_dir[gq, blocks<b]) + k
            kd_cum = np.concatenate(
                [np.zeros((NK2, 1), dtype=np.int64),
                 np.cumsum(K_dir, axis=1)], axis=1)
            gg_first = (np.arange(NK2) // NQ) * NW
            dmask = rank < K_dir[gq, blk]
            dcol = (chunkbase[gq] + kd_cum[gq, blk]
                    - kd_cum[gq, gg_first[gq]] + rank)[dmask]
            dslot = slot[dmask]
            dridx = ridx[dmask]
            # initialize all direct cells to the quarter's zero row
            for k2 in range(NK2):
                q = k2 % NQ
                if zero_ridx[q] < 0:
                    continue
                c0 = chunkbase[k2]
                nd = ND_gq[k2]
                if nd == 0:
                    continue
                cols = np.arange(c0, c0 + nd)
                wrapped[:, (8 * cols[:, None] +
                            np.arange(8)[None, :]).ravel()] = np.int16(
                    zero_ridx[q])
            wrapped[dslot % 16, 8 * dcol + dslot // 16] = dridx
        # pair chunks
        left, pos = pairpos[c]
        j = pos // P
        p = pos % P
        lgq = gq[left]
        paircol = np.searchsorted(
            union, (lgq * JMAX + j) * NBLK + blk[left])
        dstl[c, p, paircol] = slot[left].astype(np.float32)
        chunkcol = chunkbase[lgq] + ND_gq[lgq] + j
        wrapped[p % 16, 8 * chunkcol + p // 16] = ridx[left]
        # trailing pair-chunk padding -> -1 (ucode trims trailing negatives:
        # no descriptors generated, no DMA bytes moved). NOTE: hangs the
        # SWDGE ring bookkeeping on HW (decode sizes the ring from the
        # untrimmed count) -- keep off unless BASS_GCN_TRIM=1.
        for k2 in (range(NK2) if os.environ.get("BASS_GCN_TRIM", "0") == "1"
                   else ()):
            n_real = int(n_gq[c, k2])
            n_slots = int(NCH_gq[k2]) * P
            if n_real >= n_slots:
                continue
            pos = np.arange(n_real, n_slots)
            pslot = pos % P
            ccol = chunkbase[k2] + ND_gq[k2] + pos // P
            wrapped[pslot % 16, 8 * ccol + pslot // 16] = -1
        idx16[c] = wrapped[pp_ % 16, :]

    # per-(gg,q) build tables
    J_t = J_gq.reshape(NGRP, NQ)
    chunkb_t = chunkbase[:-1].reshape(NGRP, NQ)
    pairs_t = []
    dirs_t = []
    for gg in range(NGRP):
        prow = []
        drow = []
        for q in range(NQ):
            k = gg * NQ + q
            sel = slice(int(pairbase[k]), int(pairbase[k + 1]))
            prow.append(tuple(zip(pair_j[sel].tolist(),
                                  pair_b[sel].tolist())))
            dd = []
            for b in range(gg * NW, min((gg + 1) * NW, NBLK)):
                dd.extend([b] * int(K_dir[k, b]))
            drow.append(tuple(dd))
        pairs_t.append(tuple(prow))
        dirs_t.append(tuple(drow))
    pairs_t = tuple(pairs_t)
    dirs_t = tuple(dirs_t)
    pairb_t = pairbase[:-1].reshape(NGRP, NQ)

    return dict(C2=C2, NBLK=NBLK, NPC=NPC, NCHT=NCHT, NPAIR=NPAIR,
                NGRP=NGRP, NCH_t=J_t, chunkb_t=chunkb_t,
                pairs_t=pairs_t, dirs_t=dirs_t, pairb_t=pairb_t,
                xT_loc=xT_loc, dinvb=dinvb, glocb=glocb, invcnt=invcnt,
                dinvrow=dinvrow, idx16=idx16, dstl=dstl)


def _build(C2, NBLK, NPC, NCHT, NPAIR, NGRP, NCH_t, chunkb_t, pairs_t,
           dirs_t, pairb_t, hb2_val, queues=(0, 1), jcap=8,
           shared_tfull=True, single_packet=True, agseg=1, debug=False):
    JMAXQ = int(NCH_t.max())              # g buffer: chunks per (gg, q)
    JGMAX = int(NCH_t.sum(axis=1).max())  # idx staging: chunks per group
    PMAXQ = max(len(pr) for row in pairs_t for pr in row)  # oh buffer: pairs
    JCAP = jcap  # chunks per dma_gather instr (SWDGE ring capacity bound)
    table_dt = F16
    nc = bacc.Bacc("TRN2", target_bir_lowering=False, debug=False,
                   num_devices=N_CORES, num_swdge_queues=4)
    xT_d = nc.dram_tensor("xT_loc", [H, NPC], BF16, kind="ExternalInput")
    idx16_d = nc.dram_tensor("idx16", [P, 8 * NCHT], I16,
                             kind="ExternalInput")
    dstl_d = nc.dram_tensor("dstl", [P, NPAIR], table_dt,
                            kind="ExternalInput")
    dinvb_d = nc.dram_tensor("dinvb", [P, NBLK], F32, kind="ExternalInput")
    dinvrow_d = nc.dram_tensor("dinvrow", [P, NBLK * P], F16,
                               kind="ExternalInput")
    glocb_d = nc.dram_tensor("glocb", [P, NBLK], F32, kind="ExternalInput")
    invcnt_d = nc.dram_tensor("invcnt", [P, GB], F32, kind="ExternalInput")
    W_d = nc.dram_tensor("Wsb", [H, 4 * H], BF16, kind="ExternalInput")
    scol_d = nc.dram_tensor("scol", [P, 4], F32, kind="ExternalInput")
    sbcol_d = nc.dram_tensor("sbcol", [P, 4], F32, kind="ExternalInput")
    b2col_d = nc.dram_tensor("b2col", [P, 4], F32, kind="ExternalInput")
    srep3_d = nc.dram_tensor("srep3", [P, H], F32, kind="ExternalInput")
    sbrep3_d = nc.dram_tensor("sbrep3", [P, H], F32, kind="ExternalInput")
    b2rep3_d = nc.dram_tensor("b2rep3", [P, H], F32, kind="ExternalInput")
    iota16_d = nc.dram_tensor("iota16", [P, P], table_dt, kind="ExternalInput")
    iota32_d = nc.dram_tensor("iota32", [P, P], F32, kind="ExternalInput")
    hW1_d = nc.dram_tensor("hW1", [H, H], F32, kind="ExternalInput")
    hb1rep_d = nc.dram_tensor("hb1rep", [P, H], F32, kind="ExternalInput")
    hW2_d = nc.dram_tensor("hW2", [H, 1], F32, kind="ExternalInput")
    out_d = nc.dram_tensor("out", [GPC, 1], F32, kind="ExternalOutput")
    hd_d = [nc.dram_tensor(f"hdump{l}", [P, NBLK * H], F32,
                           kind="ExternalOutput")
            for l in range(4)] if debug else None
    td_d = (nc.dram_tensor("tdump", [P, NBLK * H], F32,
                           kind="ExternalOutput") if debug else None)

    NPQ = NPC // GB
    QRNG = N_CORES * NPQ
    t_loc = [[nc.dram_tensor(f"t_loc{l}_{q}", [NPQ, H], table_dt)
              for q in range(GB)] for l in range(4)]
    tf_kw = {"addr_space": "Shared"} if shared_tfull else {}
    T_full = [nc.dram_tensor(f"T_full{l}", [N_CORES * NPC, H], table_dt,
                             **tf_kw)
              for l in range(4)]

    with tile.TileContext(nc) as tc:
        with (
            tc.tile_pool(name="persist", bufs=1) as pp,
            tc.tile_pool(name="stagea", bufs=3) as sap,
            tc.tile_pool(name="streamg", bufs=2) as spg,
            tc.tile_pool(name="streamo", bufs=2) as spo,
            tc.tile_pool(name="idxs", bufs=3) as spi,
            tc.tile_pool(name="pool2", bufs=1) as wp2,
            tc.tile_pool(name="psum_agg", bufs=1, space="PSUM") as psagg_tp,
            tc.tile_pool(name="psum_a", bufs=2, space="PSUM") as psa_tp,
            tc.tile_pool(name="psum_p", bufs=1, space="PSUM") as psp_tp,
        ):
            h_sb = pp.tile([P, NBLK * H], BF16)
            t_sb = pp.tile([P, NBLK * H], table_dt)
            dstl = pp.tile([P, NPAIR], table_dt)
            dinvb = pp.tile([P, NBLK], F32)
            glocb = pp.tile([P, NBLK], F32)
            invcnt = pp.tile([P, GB], F32)
            W_sb = pp.tile([H, 4 * H], BF16)
            scol = pp.tile([P, 4], F32)
            sbcol = pp.tile([P, 4], F32)
            b2col = pp.tile([P, 4], F32)
            srep3 = pp.tile([P, H], F32)
            sbrep3 = pp.tile([P, H], F32)
            b2rep3 = pp.tile([P, H], F32)
            iota16 = pp.tile([P, P], table_dt)
            iota32 = pp.tile([P, P], F32)
            hW1_sb = pp.tile([H, H], F32)
            hb1rep = pp.tile([P, H], F32)
            hW2_sb = pp.tile([H, 1], F32)
            ident = pp.tile([P, P], F32)
            ident16 = pp.tile([P, P], table_dt)
            z2all = pp.tile([1, GPC], F32)
            for sb, d in [(dstl, dstl_d),
                          (dinvb, dinvb_d),
                          (glocb, glocb_d),
                          (invcnt, invcnt_d), (W_sb, W_d),
                          (scol, scol_d), (sbcol, sbcol_d), (b2col, b2col_d),
                          (srep3, srep3_d), (sbrep3, sbrep3_d),
                          (b2rep3, b2rep3_d),
                          (iota16, iota16_d), (iota32, iota32_d),
                          (hW1_sb, hW1_d), (hb1rep, hb1rep_d),
                          (hW2_sb, hW2_d)]:
                nc.sync.dma_start(sb[:], d[:])
            make_identity(nc, ident[:])
            nc.vector.tensor_copy(ident16[:], ident[:])
            # per-quarter x load so quarter-0 t-builds start early
            NPQ_ = NPC // GB
            for q in range(GB):
                nc.sync.dma_start(h_sb[:, q * NPQ_:(q + 1) * NPQ_],
                                  xT_d[:, q * NPQ_:(q + 1) * NPQ_])

            # zero-fill both buffer instances of each gather tile so slots
            # skipped by trailing-negative idx trimming read finite data
            for q in range(NQ):
                for _ in range(2):
                    gz = spg.tile([P, JMAXQ * H], table_dt, name=f"g{q}")
                    nc.vector.memset(gz[:], 0.0)

            ps_st = [psagg_tp.tile([P, P], F32, space="PSUM", name=f"psagg{s}")
                     for s in range(NW)]

            # hoisted num_idxs registers (one MOVE per distinct value)
            nidx_regs = {}

            def nidx_reg(v):
                if v not in nidx_regs:
                    nidx_regs[v] = nc.gpsimd.to_reg(v)
                return nidx_regs[v]

            # round-robin gather queue assignment
            qstate = [0]

            def next_queue():
                q = queues[qstate[0] % len(queues)]
                qstate[0] += 1
                return q

            def emit_gather_parts(gg, T_l):
                """Per-quarter gathers (split to fit the SWDGE descriptor
                ring) + one one-hot build per quarter, for a block group."""
                parts = []
                cg0 = int(chunkb_t[gg, 0])
                jg = int(NCH_t[gg].sum())
                idxg = spi.tile([P, 8 * JGMAX], I16, name="idxg")
                nc.sync.dma_start(idxg[:, 0:8 * jg],
                                  idx16_d[:, 8 * cg0:8 * (cg0 + jg)])
                for q in range(NQ):
                    J = int(NCH_t[gg, q])
                    NP = len(pairs_t[gg][q])
                    if J == 0:
                        continue
                    c0 = int(chunkb_t[gg, q])
                    p0 = int(pairb_t[gg, q])
                    g = spg.tile([P, JMAXQ * H], table_dt, name=f"g{q}")
                    oh = (spo.tile([P, PMAXQ * P], table_dt, name=f"oh{q}")
                          if NP > 0 else None)
                    gap = g[:]
                    # balanced split: ceil(J/JCAP) near-equal pieces
                    nsplit = -(-J // JCAP)
                    done = 0
                    for i in range(nsplit):
                        Jp = J // nsplit + (1 if i < J % nsplit else 0)
                        cc = c0 + done
                        out3 = bass.AP(gap.tensor,
                                       gap.offset + done * H,
                                       [gap.ap[0], [H, Jp], [1, H]])
                        nc.gpsimd.dma_gather(
                            out_ap=out3,
                            in_ap=T_l[q * QRNG:(q + 1) * QRNG, :],
                            idxs_ap=idxg[:, 8 * (cc - cg0):
                                         8 * (cc - cg0 + Jp)],
                            num_idxs=P * Jp,
                            num_idxs_reg=nidx_reg(P * Jp),
                            elem_size=H,
                            single_packet=single_packet,
                            queue_num=next_queue(),
                        )
                        done += Jp
                    if NP > 0:
                        oh_ap = oh[:]
                        oh3 = bass.AP(oh_ap.tensor, oh_ap.offset,
                                      [oh_ap.ap[0], [P, NP], [1, P]])
                        ia = iota16[:]
                        iota3 = bass.AP(ia.tensor, ia.offset,
                                        [ia.ap[0], [0, NP], ia.ap[1]])
                        nc.vector.tensor_tensor(
                            out=oh3,
                            in0=dstl[:, p0:p0 + NP].to_broadcast([P, NP, P]),
                            in1=iota3, op=mybir.AluOpType.is_equal)
                    parts.append((q, g, oh))
                return parts

            def emit_t_block(l, b):
                # t_l[block b] = dinv * (hT[block b]^T @ W_l), into t_loc[l]
                # hT block is [h, node]; lhsT = hT -> out [node, h'].
                ls_t = slice(l * H, (l + 1) * H)
                tps = psa_tp.tile([P, H], F32, space="PSUM", name="tps")
                nc.tensor.matmul(tps[:], lhsT=h_sb[:, b * H:(b + 1) * H],
                                 rhs=W_sb[:, ls_t],
                                 start=True, stop=True, skip_group_check=True)
                nc.scalar.activation(t_sb[:, b * H:(b + 1) * H], tps[:],
                                     mybir.ActivationFunctionType.Copy,
                                     scale=dinvb[:, b:b + 1])
                q, bq = divmod(b, NBLK // GB)
                nc.sync.dma_start(t_loc[l][q][bq * P:(bq + 1) * P, :],
                                  t_sb[:, b * H:(b + 1) * H])
                if debug and l == 0:
                    tf = sap.tile([P, H], F32, name="tdmp")
                    nc.vector.tensor_copy(tf[:], t_sb[:, b * H:(b + 1) * H])
                    nc.sync.dma_start(td_d[:, b * H:(b + 1) * H], tf[:])

            C2b = NBLK // GB   # blocks per pool quarter
            NPS = NPQ // agseg
            SRNG = N_CORES * NPS
            BPS = C2b // agseg  # blocks per AllGather segment
            NSEG = GB * agseg

            def emit_ag(l, seg):
                q, s = divmod(seg, agseg)
                base = q * QRNG + s * SRNG
                nc.gpsimd.collective_compute(
                    "AllGather", mybir.AluOpType.bypass,
                    replica_groups=[list(range(N_CORES))],
                    ins=[t_loc[l][q][s * NPS:(s + 1) * NPS, :]],
                    outs=[T_full[l][base:base + SRNG, :]])

            with nc.named_scope("stageA0"):
                nq_ = 0
                for b in range(NBLK):
                    emit_t_block(0, b)
                    while nq_ < NSEG and b >= (nq_ + 1) * BPS - 1:
                        emit_ag(0, nq_)
                        nq_ += 1

            def emit_head(gb, pps):
                pooled = wp2.tile([P, H], F32, name="pooled")
                nc.vector.tensor_scalar(pooled[:], pps[:],
                                        invcnt[:, gb:gb + 1], None,
                                        mybir.AluOpType.mult)
                # head: relu(pooled @ hW1 + hb1) @ hW2 + hb2
                trp = psp_tp.tile([P, H], F32, space="PSUM", name="pA")
                nc.tensor.transpose(out=trp[:], in_=pooled[:],
                                    identity=ident[:])
                poolT = wp2.tile([P, H], F32, name="poolT")
                nc.scalar.copy(poolT[:], trp[:])
                z1ps = psp_tp.tile([P, H], F32, space="PSUM", name="pA")
                nc.tensor.matmul(z1ps[:], lhsT=poolT[:], rhs=hW1_sb[:],
                                 start=True, stop=True,
                                 skip_group_check=True)
                r1 = wp2.tile([P, H], F32, name="r1")
                nc.vector.tensor_tensor(out=r1[:], in0=z1ps[:],
                                        in1=hb1rep[:],
                                        op=mybir.AluOpType.add)
                nc.scalar.activation(r1[:], r1[:],
                                     mybir.ActivationFunctionType.Relu)
                tr2 = psp_tp.tile([P, H], F32, space="PSUM", name="pA")
                nc.tensor.transpose(out=tr2[:], in_=r1[:], identity=ident[:])
                r1T = wp2.tile([P, H], F32, name="r1T")
                nc.scalar.copy(r1T[:], tr2[:])
                z2full = psp_tp.tile([P, P], F32, space="PSUM", name="pA")
                z2ps = z2full[0:1, :]
                nc.tensor.matmul(z2ps[:], lhsT=hW2_sb[:], rhs=r1T[:],
                                 start=True, stop=True,
                                 skip_group_check=True)
                nc.vector.tensor_scalar(
                    z2all[0:1, gb * P:(gb + 1) * P], z2ps[:],
                    float(hb2_val), None, mybir.AluOpType.add)

            for l in range(4):
                flip = l < 3
                with nc.named_scope(f"agg{l}"):
                    nq_ = 0
                    for gg in range(NGRP):
                        blocks = list(range(gg * NW, min((gg + 1) * NW, NBLK)))
                        if flip:
                            # per-group slice of the post-sum dinv[dst] rows
                            dvr = sap.tile([P, NW * P], F16, name="dvr")
                            nc.sync.dma_start(
                                dvr[:, 0:len(blocks) * P],
                                dinvrow_d[:, gg * NW * P:
                                          (gg * NW + len(blocks)) * P])
                        parts = emit_gather_parts(gg, T_full[l])
                        rem = {b: sum(1 for q in range(NQ)
                                      for (_, bb) in pairs_t[gg][q]
                                      if bb == b)
                               + sum(1 for q in range(NQ)
                                     for bb in dirs_t[gg][q]
                                     if bb == b)
                               for b in blocks}
                        for st, b in enumerate(blocks):
                            # self-loop: plain identity (table rows already
                            # carry dinv[src]; dinv[dst] applied post-sum)
                            tblk = t_sb[:, b * H:(b + 1) * H]
                            ps = ps_st[st]
                            if flip:
                                nc.tensor.matmul(ps[:], lhsT=tblk,
                                                 rhs=ident16[:], start=True,
                                                 stop=(rem[b] == 0),
                                                 skip_group_check=True)
                            else:
                                nc.tensor.matmul(ps[:], lhsT=ident16[:],
                                                 rhs=tblk, start=True,
                                                 stop=(rem[b] == 0),
                                                 skip_group_check=True)
                        for (q, g, oh) in parts:
                            ndir = len(dirs_t[gg][q])
                            for jd, b in enumerate(dirs_t[gg][q]):
                                st = b - gg * NW
                                ps = ps_st[st]
                                rem[b] -= 1
                                if flip:
                                    nc.tensor.matmul(
                                        ps[:], lhsT=g[:, jd * H:(jd + 1) * H],
                                        rhs=ident16[:],
                                        start=False, stop=(rem[b] == 0),
                                        skip_group_check=True)
                                else:
                                    nc.tensor.matmul(
                                        ps[:], lhsT=ident16[:],
                                        rhs=g[:, jd * H:(jd + 1) * H],
                                        start=False, stop=(rem[b] == 0),
                                        skip_group_check=True)
                            for k, (j, b) in enumerate(pairs_t[gg][q]):
                                st = b - gg * NW
                                ps = ps_st[st]
                                rem[b] -= 1
                                jj = ndir + j
                                if flip:
                                    nc.tensor.matmul(
                                        ps[:], lhsT=g[:, jj * H:(jj + 1) * H],
                                        rhs=oh[:, k * P:(k + 1) * P],
                                        start=False, stop=(rem[b] == 0),
                                        skip_group_check=True)
                                else:
                                    nc.tensor.matmul(
                                        ps[:], lhsT=oh[:, k * P:(k + 1) * P],
                                        rhs=g[:, jj * H:(jj + 1) * H],
                                        start=False, stop=(rem[b] == 0),
                                        skip_group_check=True)
                        for st, b in enumerate(blocks):
                            ps = ps_st[st]
                            if flip:
                                # X = S * dinv[dst] (per-column), then
                                # h = relu(s*X + s*b) + b2, per-partition
                                xs = wp2.tile([P, H], F32, name=f"xs{st}")
                                nc.vector.tensor_tensor(
                                    out=xs[:], in0=ps[:],
                                    in1=dvr[:, st * P:(st + 1) * P],
                                    op=mybir.AluOpType.mult)
                                nc.scalar.activation(
                                    h_sb[:, b * H:(b + 1) * H], xs[:],
                                    mybir.ActivationFunctionType.Relu,
                                    bias=sbcol[:, l:l + 1],
                                    scale=scol[:, l:l + 1])
                                nc.scalar.activation(
                                    h_sb[:, b * H:(b + 1) * H],
                                    h_sb[:, b * H:(b + 1) * H],
                                    mybir.ActivationFunctionType.Identity,
                                    bias=b2col[:, l:l + 1])
                                if debug:
                                    hf = sap.tile([P, H], F32, name="hdmp")
                                    nc.vector.tensor_copy(
                                        hf[:], h_sb[:, b * H:(b + 1) * H])
                                    nc.sync.dma_start(
                                        hd_d[l][:, b * H:(b + 1) * H], hf[:])
                                emit_t_block(l + 1, b)
                            else:
                                # e = dinv[dst]_p * S, then BN epilogue with
                                # per-channel (column) constants
                                e0 = wp2.tile([P, H], F32, name=f"e0_{st}")
                                e1 = wp2.tile([P, H], F32, name=f"e1_{st}")
                                nc.vector.tensor_scalar(
                                    e0[:], ps[:], dinvb[:, b:b + 1], None,
                                    mybir.AluOpType.mult)
                                nc.vector.tensor_tensor(
                                    out=e1[:], in0=e0[:], in1=srep3[:],
                                    op=mybir.AluOpType.mult)
                                nc.vector.tensor_tensor(
                                    out=e0[:], in0=e1[:], in1=sbrep3[:],
                                    op=mybir.AluOpType.add)
                                nc.scalar.activation(
                                    e1[:], e0[:],
                                    mybir.ActivationFunctionType.Relu)
                                nc.vector.tensor_tensor(
                                    out=h_sb[:, b * H:(b + 1) * H],
                                    in0=e1[:], in1=b2rep3[:],
                                    op=mybir.AluOpType.add)
                                if debug:
                                    hf = sap.tile([P, H], F32, name="hdmp")
                                    nc.vector.tensor_copy(
                                        hf[:], h_sb[:, b * H:(b + 1) * H])
                                    nc.sync.dma_start(
                                        hd_d[l][:, b * H:(b + 1) * H], hf[:])
                                # fused global-mean-pool accumulation
                                gb, kk = divmod(b, C2)
                                ohp = wp2.tile([P, P], BF16, name="ohp")
                                nc.vector.tensor_tensor(
                                    out=ohp[:],
                                    in0=glocb[:, b:b + 1].to_broadcast([P, P]),
                                    in1=iota32[:],
                                    op=mybir.AluOpType.is_equal)
                                pps = psp_tp.tile([P, H], F32, space="PSUM",
                                                  name="pps")
                                nc.tensor.matmul(
                                    pps[:], lhsT=ohp[:],
                                    rhs=h_sb[:, b * H:(b + 1) * H],
                                    start=(kk == 0), stop=(kk == C2 - 1),
                                    skip_group_check=True)
                                if kk == C2 - 1:
                                    emit_head(gb, pps)
                        if flip:
                            last_b = blocks[-1]
                            while nq_ < NSEG and last_b >= (nq_ + 1) * BPS - 1:
                                emit_ag(l + 1, nq_)
                                nq_ += 1

            with nc.named_scope("pool"):
                nc.sync.dma_start(out_d[:, 0:1], z2all[0:1, :])

    nc.compile()
    return nc


def kernel(**inputs):
    global LAST_EXEC_NS
    x = np.ascontiguousarray(np.asarray(inputs["x"], dtype=np.float32))
    ei = np.asarray(inputs["edge_index"]).astype(np.int64)
    batch = np.asarray(inputs["batch"]).astype(np.int64)
    Ws = np.asarray(inputs["Ws"], dtype=np.float32)
    bs = np.asarray(inputs["bs"], dtype=np.float32)
    gammas = np.asarray(inputs["gammas"], dtype=np.float32)
    betas = np.asarray(inputs["betas"], dtype=np.float32)
    bn_means = np.asarray(inputs["bn_means"], dtype=np.float32)
    bn_vars = np.asarray(inputs["bn_vars"], dtype=np.float32)
    hW1 = np.asarray(inputs["hW1"], dtype=np.float32)
    hb1 = np.asarray(inputs["hb1"], dtype=np.float32)
    hW2 = np.asarray(inputs["hW2"], dtype=np.float32)
    hb2 = np.asarray(inputs["hb2"], dtype=np.float32)

    src, dst = ei[0], ei[1]
    N = x.shape[0]
    deg = np.bincount(dst, minlength=N).astype(np.float64) + 1.0
    dinv = (1.0 / np.sqrt(deg)).astype(np.float32)

    fill_min = float(os.environ.get("BASS_GCN_FILL", "0"))
    agseg = int(os.environ.get("BASS_GCN_AGSEG", "1"))
    # segment split requires whole blocks per segment
    NPQ_chk = None  # C2 known only after preprocess; re-checked below
    meta = _preprocess(x, src, dst, batch, dinv, fill_min=fill_min,
                       agseg=agseg)
    if meta["C2"] % agseg != 0:
        agseg = 1
        meta = _preprocess(x, src, dst, batch, dinv, fill_min=fill_min,
                           agseg=1)
    C2, NBLK, NPC, NCHT, NPAIR, NGRP = (meta[k] for k in
                                        ("C2", "NBLK", "NPC", "NCHT",
                                         "NPAIR", "NGRP"))

    debug = os.environ.get("BASS_GCN_DEBUG", "") == "1"
    queues = tuple(int(v) for v in
                   os.environ.get("BASS_GCN_QUEUES", "0,1,2,3").split(","))
    jcap = int(os.environ.get("BASS_GCN_JCAP", "17"))
    shared_tfull = os.environ.get("BASS_GCN_SHARED", "1") == "1"
    single_packet = os.environ.get("BASS_GCN_SP", "0") == "1"
    key = (C2, NBLK, NPC, NCHT, NPAIR, NGRP, meta["pairs_t"],
           meta["dirs_t"],
           tuple(meta["NCH_t"].ravel().tolist()), float(hb2[0]),
           queues, jcap, shared_tfull, single_packet, agseg, debug)
    if key not in _CACHE:
        _CACHE[key] = _build(C2, NBLK, NPC, NCHT, NPAIR, NGRP,
                             meta["NCH_t"], meta["chunkb_t"],
                             meta["pairs_t"], meta["dirs_t"],
                             meta["pairb_t"],
                             float(hb2[0]), queues=queues, jcap=jcap,
                             shared_tfull=shared_tfull,
                             single_packet=single_packet, agseg=agseg,
                             debug=debug)
    nc = _CACHE[key]

    bf16 = mybir.dt.np(BF16)
    # replicated constant arrays
    s_l = gammas / np.sqrt(bn_vars + BN_EPS)            # [4, H]
    b2_l = betas - bn_means * s_l                        # [4, H]
    sb_l = s_l * bs                                      # [4, H]
    Wsb = np.ascontiguousarray(
        Ws.transpose(1, 0, 2).reshape(H, 4 * H)).astype(bf16)
    scol = np.ascontiguousarray(s_l.T)                   # [H, 4]
    sbcol = np.ascontiguousarray(sb_l.T)
    b2col = np.ascontiguousarray(b2_l.T)
    srep3 = np.broadcast_to(s_l[3][None, :], (P, H)).copy()
    sbrep3 = np.broadcast_to(sb_l[3][None, :], (P, H)).copy()
    b2rep3 = np.broadcast_to(b2_l[3][None, :], (P, H)).copy()
    iota16 = np.broadcast_to(np.arange(P, dtype=np.float16)[None, :],
                             (P, P)).copy()
    iota32 = iota16.astype(np.float32)
    hb1rep = np.broadcast_to(hb1[None, :], (P, H)).copy()

    in_maps = []
    for c in range(N_CORES):
        in_maps.append({
            "xT_loc": meta["xT_loc"][c].astype(bf16),
            "idx16": meta["idx16"][c],
            "dstl": meta["dstl"][c].astype(np.float16),
            "dinvb": meta["dinvb"][c],
            "dinvrow": meta["dinvrow"][c].astype(np.float16),
            "glocb": meta["glocb"][c],
            "invcnt": meta["invcnt"][c],
            "Wsb": Wsb, "scol": scol, "sbcol": sbcol, "b2col": b2col,
            "srep3": srep3, "sbrep3": sbrep3, "b2rep3": b2rep3,
            "iota16": iota16, "iota32": iota32,
            "hW1": hW1, "hb1rep": hb1rep, "hW2": hW2,
        })

    trace = os.environ.get("BASS_GCN_TRACE", "") == "1"
    if trace:
        bass_utils.upload_artifacts = lambda tmpdir: "local://" + tmpdir
        try:
            import sys, types
            if "antenv.axon_hooks" not in sys.modules:
                mod = types.ModuleType("antenv.axon_hooks")
                _h = [None]
                mod.set_axon_ntff_profile_hook = lambda h: _h.__setitem__(0, h)
                mod.get_axon_ntff_profile_hook = lambda: _h[0]
                sys.modules["antenv.axon_hooks"] = mod
                import antenv
                antenv.axon_hooks = mod
                from trn_agent_boot.trn_boot import _ntff_profile_via_ctypes
                mod.set_axon_ntff_profile_hook(
                    _ntff_profile_via_ctypes("/opt/axon/libaxon_pjrt.so"))
        except Exception as e:
            print(f"NTFF hook registration failed: {e}")
    res = bass_utils.run_bass_kernel_spmd(nc, in_maps, list(range(N_CORES)),
                                          trace=trace)
    LAST_EXEC_NS = res.exec_time_ns
    if res.exec_time_ns is not None:
        print(f"HW exec time: {res.exec_time_ns} ns")

    if debug:
        kernel.DEBUG_RES = res.results
        kernel.DEBUG_META = meta
    out = np.concatenate([res.results[c]["out"] for c in range(N_CORES)],
                         axis=0).astype(np.float32)
    return out


# revision 43
# speedup vs baseline: 1.0721x; 1.0574x over previous
"""GCN (4x GCNConv + eval BN + ReLU, global mean pool, 2-layer MLP head) on 8
Trainium2 NeuronCores via Bass/Tile.

Sharding: data-parallel over graphs. 4096 graphs -> 8 cores x 512 contiguous
graphs (batch is sorted). Within a core the 512 graphs form 4 pool groups of
128 graphs; each group's nodes are padded to a multiple of 128 rows so pooling
blocks align with node blocks. Edges live on the core owning their dst node.

Per layer (all on device):
  tt = dinv * (h_local @ W_l)           per-core shard, f16 table
  AllGather tt across the 8 cores       (the only collective)
  S[v]   = sum_{e: dst=v} tt[src_e] + tt[v]      (one-hot scatter matmuls,
                                                  PURE 0/1 one-hots + a plain
                                                  identity for the self loop)
  agg[v] = dinv[v] * S[v]                        (per-column scale, folded
                                                  into the epilogue)
  h = BN_l(relu(agg + b_l))
Key design points vs the original baseline:

* Edge rows fetched with dma_gather (SWDGE) by int16 index, split by table
  QUARTER. One gather instruction per (group, quarter) stream (JCAP=17,
  single_packet=False -- single_packet caps an instruction at 64 ring
  descriptors/engine = 8 chunks; per-desc packets lift that to the ring
  capacity of 256). Queue assignment round-robins BASS_GCN_QUEUES (default
  "0,1,2,3"); queue q runs on Q7 core pair (2q, 2q+1). Per-instruction cost
  is dominated by per-ring drain backpressure, so keeping all 4 rings evenly
  loaded beats any subset.
* num_idxs registers are hoisted (one MOVE per distinct value, not one per
  gather instruction).
* The one-hot scatter matrices are built with a SINGLE is_equal op per
  (group, quarter) — the old per-pair dstw MULTIPLY op is gone. The symmetric
  normalization's dst factor dinv[dst] is applied post-sum as a per-column
  multiply of the PSUM block against a host-precomputed dinvrow constant.
  The self-loop term is a plain constant-identity matmul (table rows already
  carry dinv[src]).
* Padding nodes carry dinv=0 in the t-build scale, so their table rows are
  exactly zero in every layer.
* Layers 0-2 run the scatter matmul "flipped" (aggT [h, node] in PSUM);
  layer 3 runs node-major so pooling sees node-major h.
* T_full (the AllGather output) is allocated in the Shared DRAM space --
  HBM-HBM AllGather is faster with Shared outputs.

All data-dependent structure is precomputed host-side into per-core meta
arrays; the chunk layout is maxed over cores so the device program is
identical across cores (SPMD).
"""

import os
import numpy as np

import concourse.bass as bass
import concourse.tile as tile
from concourse import mybir, bacc, bass_utils
from concourse.masks import make_identity

P = 128
H = 128
N_CORES = 8
N_GRAPHS = 4096
GPC = N_GRAPHS // N_CORES      # graphs per core
GB = 4                         # pool groups (of 128 graphs) per core
NQ = 4                         # table quarters (int16 index range)
BN_EPS = 1e-5
NW = 4                         # blocks per gather group / PSUM streams

F32 = mybir.dt.float32
F16 = mybir.dt.float16
BF16 = mybir.dt.bfloat16
I32 = mybir.dt.int32
I16 = mybir.dt.int16

LAST_EXEC_NS = None
_CACHE = {}


def _table_row_fn(NPC, NPQ, QRNG, seg):
    """Table layout: [quarter][segment][core][row]. seg AllGather segments
    per quarter keep each segment's AllGather output rank-contiguous."""
    NPS = NPQ // seg
    SRNG = N_CORES * NPS

    def tr(gid):
        c = gid // NPC
        i = gid % NPC
        iq = i % NPQ
        return (i // NPQ) * QRNG + (iq // NPS) * SRNG + c * NPS + (iq % NPS)
    return tr


def _preprocess(x, src, dst, batch, dinv, fill_min=0.0, agseg=1):
    """Host-side sharding: node remap + per-core padded meta arrays.

    fill_min > 0 enables "direct" chunks: per (group, quarter, block), round k
    takes at most one edge per dst slot (slot == partition == dst), while the
    mean fill over cores stays >= fill_min. Direct chunks need no one-hot --
    the device accumulates them with a constant-identity matmul; holes point
    at a guaranteed-zero table row (a padding row of the quarter).
    Leftover edges go to the usual one-hot pair chunks.
    """
    N = x.shape[0]
    graph_start = np.searchsorted(batch, np.arange(N_GRAPHS + 1))
    seg_rows = np.zeros((N_CORES, GB), dtype=np.int64)
    for c in range(N_CORES):
        for g in range(GB):
            g0 = c * GPC + g * P
            seg_rows[c, g] = graph_start[g0 + P] - graph_start[g0]
    C2 = int(np.ceil(seg_rows.max() / P))     # node blocks per pool group
    NBLK = GB * C2                            # node blocks per core
    NPC = NBLK * P                            # padded nodes per core
    NGRP = (NBLK + NW - 1) // NW

    newid = np.zeros(N, dtype=np.int64)
    for c in range(N_CORES):
        for g in range(GB):
            g0 = c * GPC + g * P
            r0, r1 = graph_start[g0], graph_start[g0 + P]
            newid[r0:r1] = c * NPC + g * C2 * P + np.arange(r1 - r0)

    xT_loc = np.zeros((N_CORES, H, NPC), dtype=np.float32)
    dinvb = np.zeros((N_CORES, P, NBLK), dtype=np.float32)   # 0 for padding
    glocb = np.full((N_CORES, P, NBLK), -1.0, dtype=np.float32)
    invcnt = np.ones((N_CORES, P, GB), dtype=np.float32)
    loc_all = newid % NPC
    core_all = newid // NPC
    for c in range(N_CORES):
        m = core_all == c
        loc = loc_all[m]
        xT_loc[c][:, loc] = x[m].T
        dinvb[c, loc % P, loc // P] = dinv[m]
        gl = (batch[m] - c * GPC).astype(np.int64)      # 0..GPC-1
        glocb[c, loc % P, loc // P] = (gl % P).astype(np.float32)
        cnt = np.zeros(GPC, dtype=np.float64)
        np.add.at(cnt, gl, 1.0)
        invcnt[c] = (1.0 / np.maximum(cnt, 1.0)).reshape(GB, P).T.astype(np.float32)

    # per-column dinv rows for the post-sum scale: [P, NBLK*P], every
    # partition holds the same row; padding columns are 0.
    dinvrow = np.zeros((N_CORES, P, NBLK * P), dtype=np.float32)
    for c in range(N_CORES):
        row = dinvb[c].T.reshape(-1)          # [NBLK*P]: col b*P+s = dinv(b,s)
        dinvrow[c] = np.broadcast_to(row, (P, NBLK * P))

    # edges grouped by (4-block group, src quarter, dst block); self-loops
    # handled by plain-identity matmuls on device. table rows live in
    # [quarter][core][row] order (quarter AllGathers).
    NPQ = NPC // GB
    QRNG = N_CORES * NPQ                      # rows per table quarter
    table_row = _table_row_fn(NPC, NPQ, QRNG, agseg)
    e_src_g = table_row(newid[src])
    e_q = e_src_g // QRNG
    e_ridx = (e_src_g % QRNG).astype(np.int16)
    e_dst_core = core_all[dst]
    e_dst_loc = loc_all[dst]

    # chunks span block boundaries within a (group, quarter): edges are
    # packed densely per (gg, q) sorted by block; each (chunk, block) overlap
    # becomes a one-hot "pair" column that masks the other blocks' slots.
    NK2 = NGRP * NQ
    e_blk = e_dst_loc // P
    e_gq = (e_blk // NW) * NQ + e_q
    e_slot = e_dst_loc % P

    # zero (padding) table row per quarter, for direct-chunk holes
    zero_ridx = np.full(NQ, -1, dtype=np.int64)
    for q in range(NQ):
        c_star = int(np.argmin(seg_rows[:, q]))
        if seg_rows[c_star, q] < C2 * P:
            gid0 = c_star * NPC + q * NPQ + seg_rows[c_star, q]
            zero_ridx[q] = table_row(np.int64(gid0)) % QRNG

    # per-core edges sorted by (gq, blk, slot), with per-slot rank
    edata = []
    KMAX = 8
    nslots_ge = np.zeros((N_CORES, NK2, NBLK, KMAX), dtype=np.int64)
    for c in range(N_CORES):
        m = e_dst_core == c
        key2 = (e_gq[m] * NBLK + e_blk[m]) * P + e_slot[m]
        order = np.argsort(key2, kind="stable")
        gq = e_gq[m][order]
        blk = e_blk[m][order]
        slot = e_slot[m][order]
        ridx = e_ridx[m][order]
        key2s = key2[order]
        first = np.concatenate([[0], np.flatnonzero(np.diff(key2s)) + 1])
        starts = np.zeros(len(key2s), dtype=np.int64)
        starts[first] = first
        starts = np.maximum.accumulate(starts)
        rank = np.arange(len(key2s)) - starts
        cnt = np.bincount(key2s, minlength=NK2 * NBLK * P).reshape(
            NK2, NBLK, P)
        for k in range(KMAX):
            nslots_ge[c, :, :, k] = (cnt > k).sum(axis=2)
        edata.append((gq, blk, slot, ridx, rank))

    # shared direct-round counts per (gq, b): mean fill over cores >= fill_min
    K_dir = np.zeros((NK2, NBLK), dtype=np.int64)
    if fill_min > 0:
        mean_fill = nslots_ge.mean(axis=0) / P          # [NK2, NBLK, KMAX]
        for k in range(KMAX):
            K_dir[(mean_fill[:, :, k] >= fill_min) & (K_dir == k)] = k + 1
        # quarters without a zero row can't host direct holes
        for q in range(NQ):
            if zero_ridx[q] < 0:
                K_dir[q::NQ, :] = 0

    # leftover (pair) edges per core/stream; shared pair chunk counts
    n_gq = np.zeros((N_CORES, NK2), dtype=np.int64)
    for c in range(N_CORES):
        gq, blk, slot, ridx, rank = edata[c]
        left = rank >= K_dir[gq, blk]
        n_gq[c] = np.bincount(gq[left], minlength=NK2)
    NCH_gq = -(-n_gq.max(axis=0) // P)                   # pair chunks
    ND_gq = np.array([[K_dir[gg * NQ + q,
                             gg * NW:min((gg + 1) * NW, NBLK)].sum()
                       for q in range(NQ)] for gg in range(NGRP)]
                     ).reshape(-1)                        # direct chunks
    J_gq = ND_gq + NCH_gq                                # total chunks
    chunkbase = np.concatenate([[0], np.cumsum(J_gq)])
    NCHT = int(chunkbase[-1])                 # total chunk columns (idx/g)

    # union pair list (gq, pair chunk j, block): encoded, sorted => (gq,j,b)
    JMAX = int(NCH_gq.max()) if NCH_gq.max() > 0 else 1
    encs = []
    pairpos = []                               # per-core leftover positions
    for c in range(N_CORES):
        gq, blk, slot, ridx, rank = edata[c]
        left = np.flatnonzero(rank >= K_dir[gq, blk])
        lgq = gq[left]
        start2 = np.concatenate(
            [[0], np.cumsum(np.bincount(lgq, minlength=NK2))])
        pos = np.arange(len(left)) - start2[lgq]
        pairpos.append((left, pos))
        encs.append((lgq * JMAX + pos // P) * NBLK + blk[left])
    union = np.unique(np.concatenate(encs))
    NPAIR = max(len(union), 1)
    pair_gq = union // (JMAX * NBLK)
    pair_j = (union // NBLK) % JMAX
    pair_b = union % NBLK
    np_gq = np.bincount(pair_gq, minlength=NK2)
    pairbase = np.concatenate([[0], np.cumsum(np_gq)])

    dstl = np.full((N_CORES, P, NPAIR), -1.0, dtype=np.float32)
    idx16 = np.zeros((N_CORES, P, 8 * NCHT), dtype=np.int16)
    pp_ = np.arange(P)
    for c in range(N_CORES):
        gq, blk, slot, ridx, rank = edata[c]
        wrapped = np.zeros((16, 8 * NCHT), dtype=np.int16)
        # direct chunks: idx position == dst slot; holes -> zero row
        if fill_min > 0:
            # chunk column of direct round k of (gq, b):
            # chunkbase[gq] + sum(K_dir[gq, blocks<b]) + k
            kd_cum = np.concatenate(
                [np.zeros((NK2, 1), dtype=np.int64),
                 np.cumsum(K_dir, axis=1)], axis=1)
            gg_first = (np.arange(NK2) // NQ) * NW
            dmask = rank < K_dir[gq, blk]
            dcol = (chunkbase[gq] + kd_cum[gq, blk]
                    - kd_cum[gq, gg_first[gq]] + rank)[dmask]
            dslot = slot[dmask]
            dridx = ridx[dmask]
            # initialize all direct cells to the quarter's zero row
            for k2 in range(NK2):
                q = k2 % NQ
                if zero_ridx[q] < 0:
                    continue
                c0 = chunkbase[k2]
                nd = ND_gq[k2]
                if nd == 0:
                    continue
                cols = np.arange(c0, c0 + nd)
                wrapped[:, (8 * cols[:, None] +
                            np.arange(8)[None, :]).ravel()] = np.int16(
                    zero_ridx[q])
            wrapped[dslot % 16, 8 * dcol + dslot // 16] = dridx
        # pair chunks
        left, pos = pairpos[c]
        j = pos // P
        p = pos % P
        lgq = gq[left]
        paircol = np.searchsorted(
            union, (lgq * JMAX + j) * NBLK + blk[left])
        dstl[c, p, paircol] = slot[left].astype(np.float32)
        chunkcol = chunkbase[lgq] + ND_gq[lgq] + j
        wrapped[p % 16, 8 * chunkcol + p // 16] = ridx[left]
        # trailing pair-chunk padding -> -1 (ucode trims trailing negatives:
        # no descriptors generated, no DMA bytes moved). NOTE: hangs the
        # SWDGE ring bookkeeping on HW (decode sizes the ring from the
        # untrimmed count) -- keep off unless BASS_GCN_TRIM=1.
        for k2 in (range(NK2) if os.environ.get("BASS_GCN_TRIM", "0") == "1"
                   else ()):
            n_real = int(n_gq[c, k2])
            n_slots = int(NCH_gq[k2]) * P
            if n_real >= n_slots:
                continue
            pos = np.arange(n_real, n_slots)
            pslot = pos % P
            ccol = chunkbase[k2] + ND_gq[k2] + pos // P
            wrapped[pslot % 16, 8 * ccol + pslot // 16] = -1
        idx16[c] = wrapped[pp_ % 16, :]

    # per-(gg,q) build tables
    J_t = J_gq.reshape(NGRP, NQ)
    chunkb_t = chunkbase[:-1].reshape(NGRP, NQ)
    pairs_t = []
    dirs_t = []
    for gg in range(NGRP):
        prow = []
        drow = []
        for q in range(NQ):
            k = gg * NQ + q
            sel = slice(int(pairbase[k]), int(pairbase[k + 1]))
            prow.append(tuple(zip(pair_j[sel].tolist(),
                                  pair_b[sel].tolist())))
            dd = []
            for b in range(gg * NW, min((gg + 1) * NW, NBLK)):
                dd.extend([b] * int(K_dir[k, b]))
            drow.append(tuple(dd))
        pairs_t.append(tuple(prow))
        dirs_t.append(tuple(drow))
    pairs_t = tuple(pairs_t)
    dirs_t = tuple(dirs_t)
    pairb_t = pairbase[:-1].reshape(NGRP, NQ)

    return dict(C2=C2, NBLK=NBLK, NPC=NPC, NCHT=NCHT, NPAIR=NPAIR,
                NGRP=NGRP, NCH_t=J_t, chunkb_t=chunkb_t,
                pairs_t=pairs_t, dirs_t=dirs_t, pairb_t=pairb_t,
                xT_loc=xT_loc, dinvb=dinvb, glocb=glocb, invcnt=invcnt,
                dinvrow=dinvrow, idx16=idx16, dstl=dstl)


def _build(C2, NBLK, NPC, NCHT, NPAIR, NGRP, NCH_t, chunkb_t, pairs_t,
           dirs_t, pairb_t, hb2_val, queues=(0, 1), jcap=8,
           shared_tfull=True, single_packet=True, agseg=1, debug=False):
    JMAXQ = int(NCH_t.max())              # g buffer: chunks per (gg, q)
    JGMAX = int(NCH_t.sum(axis=1).max())  # idx staging: chunks per group
    PMAXQ = max(len(pr) for row in pairs_t for pr in row)  # oh buffer: pairs
    JCAP = jcap  # chunks per dma_gather instr (SWDGE ring capacity bound)
    table_dt = F16
    nc = bacc.Bacc("TRN2", target_bir_lowering=False, debug=False,
                   num_devices=N_CORES, num_swdge_queues=4)
    xT_d = nc.dram_tensor("xT_loc", [H, NPC], BF16, kind="ExternalInput")
    idx16_d = nc.dram_tensor("idx16", [P, 8 * NCHT], I16,
                             kind="ExternalInput")
    dstl_d = nc.dram_tensor("dstl", [P, NPAIR], table_dt,
                            kind="ExternalInput")
    dinvb_d = nc.dram_tensor("dinvb", [P, NBLK], F32, kind="ExternalInput")
    dinvrow_d = nc.dram_tensor("dinvrow", [P, NBLK * P], F16,
                               kind="ExternalInput")
    glocb_d = nc.dram_tensor("glocb", [P, NBLK], F32, kind="ExternalInput")
    invcnt_d = nc.dram_tensor("invcnt", [P, GB], F32, kind="ExternalInput")
    W_d = nc.dram_tensor("Wsb", [H, 4 * H], BF16, kind="ExternalInput")
    scol_d = nc.dram_tensor("scol", [P, 4], F32, kind="ExternalInput")
    sbcol_d = nc.dram_tensor("sbcol", [P, 4], F32, kind="ExternalInput")
    b2col_d = nc.dram_tensor("b2col", [P, 4], F32, kind="ExternalInput")
    srep3_d = nc.dram_tensor("srep3", [P, H], F32, kind="ExternalInput")
    sbrep3_d = nc.dram_tensor("sbrep3", [P, H], F32, kind="ExternalInput")
    b2rep3_d = nc.dram_tensor("b2rep3", [P, H], F32, kind="ExternalInput")
    iota16_d = nc.dram_tensor("iota16", [P, P], table_dt, kind="ExternalInput")
    iota32_d = nc.dram_tensor("iota32", [P, P], F32, kind="ExternalInput")
    hW1_d = nc.dram_tensor("hW1", [H, H], F32, kind="ExternalInput")
    hb1rep_d = nc.dram_tensor("hb1rep", [P, H], F32, kind="ExternalInput")
    hW2_d = nc.dram_tensor("hW2", [H, 1], F32, kind="ExternalInput")
    out_d = nc.dram_tensor("out", [GPC, 1], F32, kind="ExternalOutput")
    hd_d = [nc.dram_tensor(f"hdump{l}", [P, NBLK * H], F32,
                           kind="ExternalOutput")
            for l in range(4)] if debug else None
    td_d = (nc.dram_tensor("tdump", [P, NBLK * H], F32,
                           kind="ExternalOutput") if debug else None)

    NPQ = NPC // GB
    QRNG = N_CORES * NPQ
    t_loc = [[nc.dram_tensor(f"t_loc{l}_{q}", [NPQ, H], table_dt)
              for q in range(GB)] for l in range(4)]
    tf_kw = {"addr_space": "Shared"} if shared_tfull else {}
    T_full = [nc.dram_tensor(f"T_full{l}", [N_CORES * NPC, H], table_dt,
                             **tf_kw)
              for l in range(4)]

    with tile.TileContext(nc) as tc:
        with (
            tc.tile_pool(name="persist", bufs=1) as pp,
            tc.tile_pool(name="stagea", bufs=3) as sap,
            tc.tile_pool(name="streamg", bufs=2) as spg,
            tc.tile_pool(name="streamo", bufs=2) as spo,
            tc.tile_pool(name="pool2", bufs=1) as wp2,
            tc.tile_pool(name="psum_agg", bufs=1, space="PSUM") as psagg_tp,
            tc.tile_pool(name="psum_a", bufs=2, space="PSUM") as psa_tp,
            tc.tile_pool(name="psum_p", bufs=1, space="PSUM") as psp_tp,
        ):
            h_sb = pp.tile([P, NBLK * H], BF16)
            t_sb = pp.tile([P, NBLK * H], table_dt)
            idx16 = pp.tile([P, 8 * NCHT], I16)
            dstl = pp.tile([P, NPAIR], table_dt)
            dinvb = pp.tile([P, NBLK], F32)
            glocb = pp.tile([P, NBLK], F32)
            invcnt = pp.tile([P, GB], F32)
            W_sb = pp.tile([H, 4 * H], BF16)
            scol = pp.tile([P, 4], F32)
            sbcol = pp.tile([P, 4], F32)
            b2col = pp.tile([P, 4], F32)
            srep3 = pp.tile([P, H], F32)
            sbrep3 = pp.tile([P, H], F32)
            b2rep3 = pp.tile([P, H], F32)
            iota16 = pp.tile([P, P], table_dt)
            iota32 = pp.tile([P, P], F32)
            hW1_sb = pp.tile([H, H], F32)
            hb1rep = pp.tile([P, H], F32)
            hW2_sb = pp.tile([H, 1], F32)
            ident = pp.tile([P, P], F32)
            ident16 = pp.tile([P, P], table_dt)
            z2all = pp.tile([1, GPC], F32)
            for sb, d in [(idx16, idx16_d), (dstl, dstl_d),
                          (dinvb, dinvb_d),
                          (glocb, glocb_d),
                          (invcnt, invcnt_d), (W_sb, W_d),
                          (scol, scol_d), (sbcol, sbcol_d), (b2col, b2col_d),
                          (srep3, srep3_d), (sbrep3, sbrep3_d),
                          (b2rep3, b2rep3_d),
                          (iota16, iota16_d), (iota32, iota32_d),
                          (hW1_sb, hW1_d), (hb1rep, hb1rep_d),
                          (hW2_sb, hW2_d)]:
                nc.sync.dma_start(sb[:], d[:])
            make_identity(nc, ident[:])
            nc.vector.tensor_copy(ident16[:], ident[:])
            # per-quarter x load so quarter-0 t-builds start early
            NPQ_ = NPC // GB
            for q in range(GB):
                nc.sync.dma_start(h_sb[:, q * NPQ_:(q + 1) * NPQ_],
                                  xT_d[:, q * NPQ_:(q + 1) * NPQ_])

            # zero-fill both buffer instances of each gather tile so slots
            # skipped by trailing-negative idx trimming read finite data
            for q in range(NQ):
                for _ in range(2):
                    gz = spg.tile([P, JMAXQ * H], table_dt, name=f"g{q}")
                    nc.vector.memset(gz[:], 0.0)

            ps_st = [psagg_tp.tile([P, P], F32, space="PSUM", name=f"psagg{s}")
                     for s in range(NW)]

            # hoisted num_idxs registers (one MOVE per distinct value)
            nidx_regs = {}

            def nidx_reg(v):
                if v not in nidx_regs:
                    nidx_regs[v] = nc.gpsimd.to_reg(v)
                return nidx_regs[v]

            # round-robin gather queue assignment
            qstate = [0]

            def next_queue():
                q = queues[qstate[0] % len(queues)]
                qstate[0] += 1
                return q

            def emit_gather_parts(gg, T_l):
                """Per-quarter gathers (split to fit the SWDGE descriptor
                ring) + one one-hot build per quarter, for a block group."""
                parts = []
                for q in range(NQ):
                    J = int(NCH_t[gg, q])
                    NP = len(pairs_t[gg][q])
                    if J == 0:
                        continue
                    c0 = int(chunkb_t[gg, q])
                    p0 = int(pairb_t[gg, q])
                    g = spg.tile([P, JMAXQ * H], table_dt, name=f"g{q}")
                    oh = (spo.tile([P, PMAXQ * P], table_dt, name=f"oh{q}")
                          if NP > 0 else None)
                    gap = g[:]
                    # balanced split: ceil(J/JCAP) near-equal pieces
                    nsplit = -(-J // JCAP)
                    done = 0
                    for i in range(nsplit):
                        Jp = J // nsplit + (1 if i < J % nsplit else 0)
                        cc = c0 + done
                        out3 = bass.AP(gap.tensor,
                                       gap.offset + done * H,
                                       [gap.ap[0], [H, Jp], [1, H]])
                        nc.gpsimd.dma_gather(
                            out_ap=out3,
                            in_ap=T_l[q * QRNG:(q + 1) * QRNG, :],
                            idxs_ap=idx16[:, 8 * cc:8 * (cc + Jp)],
                            num_idxs=P * Jp,
                            num_idxs_reg=nidx_reg(P * Jp),
                            elem_size=H,
                            single_packet=single_packet,
                            queue_num=next_queue(),
                        )
                        done += Jp
                    if NP > 0:
                        oh_ap = oh[:]
                        oh3 = bass.AP(oh_ap.tensor, oh_ap.offset,
                                      [oh_ap.ap[0], [P, NP], [1, P]])
                        ia = iota16[:]
                        iota3 = bass.AP(ia.tensor, ia.offset,
                                        [ia.ap[0], [0, NP], ia.ap[1]])
                        nc.vector.tensor_tensor(
                            out=oh3,
                            in0=dstl[:, p0:p0 + NP].to_broadcast([P, NP, P]),
                            in1=iota3, op=mybir.AluOpType.is_equal)
                    parts.append((q, g, oh))
                return parts

            def emit_t_block(l, b):
                # t_l[block b] = dinv * (hT[block b]^T @ W_l), into t_loc[l]
                # hT block is [h, node]; lhsT = hT -> out [node, h'].
                ls_t = slice(l * H, (l + 1) * H)
                tps = psa_tp.tile([P, H], F32, space="PSUM", name="tps")
                nc.tensor.matmul(tps[:], lhsT=h_sb[:, b * H:(b + 1) * H],
                                 rhs=W_sb[:, ls_t],
                                 start=True, stop=True, skip_group_check=True)
                nc.scalar.activation(t_sb[:, b * H:(b + 1) * H], tps[:],
                                     mybir.ActivationFunctionType.Copy,
                                     scale=dinvb[:, b:b + 1])
                q, bq = divmod(b, NBLK // GB)
                nc.sync.dma_start(t_loc[l][q][bq * P:(bq + 1) * P, :],
                                  t_sb[:, b * H:(b + 1) * H])
                if debug and l == 0:
                    tf = sap.tile([P, H], F32, name="tdmp")
                    nc.vector.tensor_copy(tf[:], t_sb[:, b * H:(b + 1) * H])
                    nc.sync.dma_start(td_d[:, b * H:(b + 1) * H], tf[:])

            C2b = NBLK // GB   # blocks per pool quarter
            NPS = NPQ // agseg
            SRNG = N_CORES * NPS
            BPS = C2b // agseg  # blocks per AllGather segment
            NSEG = GB * agseg

            def emit_ag(l, seg):
                q, s = divmod(seg, agseg)
                base = q * QRNG + s * SRNG
                nc.gpsimd.collective_compute(
                    "AllGather", mybir.AluOpType.bypass,
                    replica_groups=[list(range(N_CORES))],
                    ins=[t_loc[l][q][s * NPS:(s + 1) * NPS, :]],
                    outs=[T_full[l][base:base + SRNG, :]])

            with nc.named_scope("stageA0"):
                nq_ = 0
                for b in range(NBLK):
                    emit_t_block(0, b)
                    while nq_ < NSEG and b >= (nq_ + 1) * BPS - 1:
                        emit_ag(0, nq_)
                        nq_ += 1

            def emit_head(gb, pps):
                pooled = wp2.tile([P, H], F32, name="pooled")
                nc.vector.tensor_scalar(pooled[:], pps[:],
                                        invcnt[:, gb:gb + 1], None,
                                        mybir.AluOpType.mult)
                # head: relu(pooled @ hW1 + hb1) @ hW2 + hb2
                trp = psp_tp.tile([P, H], F32, space="PSUM", name="pA")
                nc.tensor.transpose(out=trp[:], in_=pooled[:],
                                    identity=ident[:])
                poolT = wp2.tile([P, H], F32, name="poolT")
                nc.scalar.copy(poolT[:], trp[:])
                z1ps = psp_tp.tile([P, H], F32, space="PSUM", name="pA")
                nc.tensor.matmul(z1ps[:], lhsT=poolT[:], rhs=hW1_sb[:],
                                 start=True, stop=True,
                                 skip_group_check=True)
                r1 = wp2.tile([P, H], F32, name="r1")
                nc.vector.tensor_tensor(out=r1[:], in0=z1ps[:],
                                        in1=hb1rep[:],
                                        op=mybir.AluOpType.add)
                nc.scalar.activation(r1[:], r1[:],
                                     mybir.ActivationFunctionType.Relu)
                tr2 = psp_tp.tile([P, H], F32, space="PSUM", name="pA")
                nc.tensor.transpose(out=tr2[:], in_=r1[:], identity=ident[:])
                r1T = wp2.tile([P, H], F32, name="r1T")
                nc.scalar.copy(r1T[:], tr2[:])
                z2full = psp_tp.tile([P, P], F32, space="PSUM", name="pA")
                z2ps = z2full[0:1, :]
                nc.tensor.matmul(z2ps[:], lhsT=hW2_sb[:], rhs=r1T[:],
                                 start=True, stop=True,
                                 skip_group_check=True)
                nc.vector.tensor_scalar(
                    z2all[0:1, gb * P:(gb + 1) * P], z2ps[:],
                    float(hb2_val), None, mybir.AluOpType.add)

            for l in range(4):
                flip = l < 3
                with nc.named_scope(f"agg{l}"):
                    nq_ = 0
                    for gg in range(NGRP):
                        blocks = list(range(gg * NW, min((gg + 1) * NW, NBLK)))
                        if flip:
                            # per-group slice of the post-sum dinv[dst] rows
                            dvr = sap.tile([P, NW * P], F16, name="dvr")
                            nc.sync.dma_start(
                                dvr[:, 0:len(blocks) * P],
                                dinvrow_d[:, gg * NW * P:
                                          (gg * NW + len(blocks)) * P])
                        parts = emit_gather_parts(gg, T_full[l])
                        rem = {b: sum(1 for q in range(NQ)
                                      for (_, bb) in pairs_t[gg][q]
                                      if bb == b)
                               + sum(1 for q in range(NQ)
                                     for bb in dirs_t[gg][q]
                                     if bb == b)
                               for b in blocks}
                        for st, b in enumerate(blocks):
                            # self-loop: plain identity (table rows already
                            # carry dinv[src]; dinv[dst] applied post-sum)
                            tblk = t_sb[:, b * H:(b + 1) * H]
                            ps = ps_st[st]
                            if flip:
                                nc.tensor.matmul(ps[:], lhsT=tblk,
                                                 rhs=ident16[:], start=True,
                                                 stop=(rem[b] == 0),
                                                 skip_group_check=True)
                            else:
                                nc.tensor.matmul(ps[:], lhsT=ident16[:],
                                                 rhs=tblk, start=True,
                                                 stop=(rem[b] == 0),
                                                 skip_group_check=True)
                        for (q, g, oh) in parts:
                            ndir = len(dirs_t[gg][q])
                            for jd, b in enumerate(dirs_t[gg][q]):
                                st = b - gg * NW
                                ps = ps_st[st]
                                rem[b] -= 1
                                if flip:
                                    nc.tensor.matmul(
                                        ps[:], lhsT=g[:, jd * H:(jd + 1) * H],
                                        rhs=ident16[:],
                                        start=False, stop=(rem[b] == 0),
                                        skip_group_check=True)
                                else:
                                    nc.tensor.matmul(
                                        ps[:], lhsT=ident16[:],
                                        rhs=g[:, jd * H:(jd + 1) * H],
                                        start=False, stop=(rem[b] == 0),
                                        skip_group_check=True)
                            for k, (j, b) in enumerate(pairs_t[gg][q]):
                                st = b - gg * NW
                                ps = ps_st[st]
                                rem[b] -= 1
                                jj = ndir + j
                                if flip:
                                    nc.tensor.matmul(
                                        ps[:], lhsT=g[:, jj * H:(jj + 1) * H],
                                        rhs=oh[:, k * P:(k + 1) * P],
                                        start=False, stop=(rem[b] == 0),
                                        skip_group_check=True)
                                else:
                                    nc.tensor.matmul(
                                        ps[:], lhsT=oh[:, k * P:(k + 1) * P],
                                        rhs=g[:, jj * H:(jj + 1) * H],
                                        start=False, stop=(rem[b] == 0),
                                        skip_group_check=True)
                        for st, b in enumerate(blocks):
                            ps = ps_st[st]
                            if flip:
                                # X = S * dinv[dst] (per-column), then
                                # h = relu(s*X + s*b) + b2, per-partition
                                xs = wp2.tile([P, H], F32, name=f"xs{st}")
                                nc.vector.tensor_tensor(
                                    out=xs[:], in0=ps[:],
                                    in1=dvr[:, st * P:(st + 1) * P],
                                    op=mybir.AluOpType.mult)
                                nc.scalar.activation(
                                    h_sb[:, b * H:(b + 1) * H], xs[:],
                                    mybir.ActivationFunctionType.Relu,
                                    bias=sbcol[:, l:l + 1],
                                    scale=scol[:, l:l + 1])
                                nc.scalar.activation(
                                    h_sb[:, b * H:(b + 1) * H],
                                    h_sb[:, b * H:(b + 1) * H],
                                    mybir.ActivationFunctionType.Identity,
                                    bias=b2col[:, l:l + 1])
                                if debug:
                                    hf = sap.tile([P, H], F32, name="hdmp")
                                    nc.vector.tensor_copy(
                                        hf[:], h_sb[:, b * H:(b + 1) * H])
                                    nc.sync.dma_start(
                                        hd_d[l][:, b * H:(b + 1) * H], hf[:])
                                emit_t_block(l + 1, b)
                            else:
                                # e = dinv[dst]_p * S, then BN epilogue with
                                # per-channel (column) constants
                                e0 = wp2.tile([P, H], F32, name=f"e0_{st}")
                                e1 = wp2.tile([P, H], F32, name=f"e1_{st}")
                                nc.vector.tensor_scalar(
                                    e0[:], ps[:], dinvb[:, b:b + 1], None,
                                    mybir.AluOpType.mult)
                                nc.vector.tensor_tensor(
                                    out=e1[:], in0=e0[:], in1=srep3[:],
                                    op=mybir.AluOpType.mult)
                                nc.vector.tensor_tensor(
                                    out=e0[:], in0=e1[:], in1=sbrep3[:],
                                    op=mybir.AluOpType.add)
                                nc.scalar.activation(
                                    e1[:], e0[:],
                                    mybir.ActivationFunctionType.Relu)
                                nc.vector.tensor_tensor(
                                    out=h_sb[:, b * H:(b + 1) * H],
                                    in0=e1[:], in1=b2rep3[:],
                                    op=mybir.AluOpType.add)
                                if debug:
                                    hf = sap.tile([P, H], F32, name="hdmp")
                                    nc.vector.tensor_copy(
                                        hf[:], h_sb[:, b * H:(b + 1) * H])
                                    nc.sync.dma_start(
                                        hd_d[l][:, b * H:(b + 1) * H], hf[:])
                                # fused global-mean-pool accumulation
                                gb, kk = divmod(b, C2)
                                ohp = wp2.tile([P, P], BF16, name="ohp")
                                nc.vector.tensor_tensor(
                                    out=ohp[:],
                                    in0=glocb[:, b:b + 1].to_broadcast([P, P]),
                                    in1=iota32[:],
                                    op=mybir.AluOpType.is_equal)
                                pps = psp_tp.tile([P, H], F32, space="PSUM",
                                                  name="pps")
                                nc.tensor.matmul(
                                    pps[:], lhsT=ohp[:],
                                    rhs=h_sb[:, b * H:(b + 1) * H],
                                    start=(kk == 0), stop=(kk == C2 - 1),
                                    skip_group_check=True)
                                if kk == C2 - 1:
                                    emit_head(gb, pps)
                        if flip:
                            last_b = blocks[-1]
                            while nq_ < NSEG and last_b >= (nq_ + 1) * BPS - 1:
                                emit_ag(l + 1, nq_)
                                nq_ += 1

            with nc.named_scope("pool"):
                nc.sync.dma_start(out_d[:, 0:1], z2all[0:1, :])

    nc.compile()
    return nc


def kernel(**inputs):
    global LAST_EXEC_NS
    x = np.ascontiguousarray(np.asarray(inputs["x"], dtype=np.float32))
    ei = np.asarray(inputs["edge_index"]).astype(np.int64)
    batch = np.asarray(inputs["batch"]).astype(np.int64)
    Ws = np.asarray(inputs["Ws"], dtype=np.float32)
    bs = np.asarray(inputs["bs"], dtype=np.float32)
    gammas = np.asarray(inputs["gammas"], dtype=np.float32)
    betas = np.asarray(inputs["betas"], dtype=np.float32)
    bn_means = np.asarray(inputs["bn_means"], dtype=np.float32)
    bn_vars = np.asarray(inputs["bn_vars"], dtype=np.float32)
    hW1 = np.asarray(inputs["hW1"], dtype=np.float32)
    hb1 = np.asarray(inputs["hb1"], dtype=np.float32)
    hW2 = np.asarray(inputs["hW2"], dtype=np.float32)
    hb2 = np.asarray(inputs["hb2"], dtype=np.float32)

    src, dst = ei[0], ei[1]
    N = x.shape[0]
    deg = np.bincount(dst, minlength=N).astype(np.float64) + 1.0
    dinv = (1.0 / np.sqrt(deg)).astype(np.float32)

    fill_min = float(os.environ.get("BASS_GCN_FILL", "0"))
    agseg = int(os.environ.get("BASS_GCN_AGSEG", "1"))
    # segment split requires whole blocks per segment
    NPQ_chk = None  # C2 known only after preprocess; re-checked below
    meta = _preprocess(x, src, dst, batch, dinv, fill_min=fill_min,
                       agseg=agseg)
    if meta["C2"] % agseg != 0:
        agseg = 1
        meta = _preprocess(x, src, dst, batch, dinv, fill_min=fill_min,
                           agseg=1)
    C2, NBLK, NPC, NCHT, NPAIR, NGRP = (meta[k] for k in
                                        ("C2", "NBLK", "NPC", "NCHT",
                                         "NPAIR", "NGRP"))

    debug = os.environ.get("BASS_GCN_DEBUG", "") == "1"
    queues = tuple(int(v) for v in
                   os.environ.get("BASS_GCN_QUEUES", "0,1,2,3").split(","))
    jcap = int(os.environ.get("BASS_GCN_JCAP", "17"))
    shared_tfull = os.environ.get("BASS_GCN_SHARED", "1") == "1"
    single_packet = os.environ.get("BASS_GCN_SP", "0") == "1"
    key = (C2, NBLK, NPC, NCHT, NPAIR, NGRP, meta["pairs_t"],
           meta["dirs_t"],
           tuple(meta["NCH_t"].ravel().tolist()), float(hb2[0]),
           queues, jcap, shared_tfull, single_packet, agseg, debug)
    if key not in _CACHE:
        _CACHE[key] = _build(C2, NBLK, NPC, NCHT, NPAIR, NGRP,
                             meta["NCH_t"], meta["chunkb_t"],
                             meta["pairs_t"], meta["dirs_t"],
                             meta["pairb_t"],
                             float(hb2[0]), queues=queues, jcap=jcap,
                             shared_tfull=shared_tfull,
                             single_packet=single_packet, agseg=agseg,
                             debug=debug)
    nc = _CACHE[key]

    bf16 = mybir.dt.np(BF16)
    # replicated constant arrays
    s_l = gammas / np.sqrt(bn_vars + BN_EPS)            # [4, H]
    b2_l = betas - bn_means * s_l                        # [4, H]
    sb_l = s_l * bs                                      # [4, H]
    Wsb = np.ascontiguousarray(
        Ws.transpose(1, 0, 2).reshape(H, 4 * H)).astype(bf16)
    scol = np.ascontiguousarray(s_l.T)                   # [H, 4]
    sbcol = np.ascontiguousarray(sb_l.T)
    b2col = np.ascontiguousarray(b2_l.T)
    srep3 = np.broadcast_to(s_l[3][None, :], (P, H)).copy()
    sbrep3 = np.broadcast_to(sb_l[3][None, :], (P, H)).copy()
    b2rep3 = np.broadcast_to(b2_l[3][None, :], (P, H)).copy()
    iota16 = np.broadcast_to(np.arange(P, dtype=np.float16)[None, :],
                             (P, P)).copy()
    iota32 = iota16.astype(np.float32)
    hb1rep = np.broadcast_to(hb1[None, :], (P, H)).copy()

    in_maps = []
    for c in range(N_CORES):
        in_maps.append({
            "xT_loc": meta["xT_loc"][c].astype(bf16),
            "idx16": meta["idx16"][c],
            "dstl": meta["dstl"][c].astype(np.float16),
            "dinvb": meta["dinvb"][c],
            "dinvrow": meta["dinvrow"][c].astype(np.float16),
            "glocb": meta["glocb"][c],
            "invcnt": meta["invcnt"][c],
            "Wsb": Wsb, "scol": scol, "sbcol": sbcol, "b2col": b2col,
            "srep3": srep3, "sbrep3": sbrep3, "b2rep3": b2rep3,
            "iota16": iota16, "iota32": iota32,
            "hW1": hW1, "hb1rep": hb1rep, "hW2": hW2,
        })

    trace = os.environ.get("BASS_GCN_TRACE", "") == "1"
    if trace:
        bass_utils.upload_artifacts = lambda tmpdir: "local://" + tmpdir
        try:
            import sys, types
            if "antenv.axon_hooks" not in sys.modules:
                mod = types.ModuleType("antenv.axon_hooks")
                _h = [None]
                mod.set_axon_ntff_profile_hook = lambda h: _h.__setitem__(0, h)
                mod.get_axon_ntff_profile_hook = lambda: _h[0]
                sys.modules["antenv.axon_hooks"] = mod
                import antenv
                antenv.axon_hooks = mod
                from trn_agent_boot.trn_boot import _ntff_profile_via_ctypes
                mod.set_axon_ntff_profile_hook(
                    _ntff_profile_via_ctypes("/opt/axon/libaxon_pjrt.so"))
        except Exception as e:
            print(f"NTFF hook registration failed: {e}")
    res = bass_utils.run_bass_kernel_spmd(nc, in_maps, list(range(N_CORES)),
                                          trace=trace)
    LAST_EXEC_NS = res.exec_time_ns
    if res.exec_time_ns is not None:
        print(f"HW exec time: {res.exec_time_ns} ns")

    if debug:
        kernel.DEBUG_RES = res.results
        kernel.DEBUG_META = meta
    out = np.concatenate([res.results[c]["out"] for c in range(N_CORES)],
                         axis=0).astype(np.float32)
    return out
